# revision 44
# baseline (speedup 1.0000x reference)
"""Bass/Trainium2 kernel for per-head attention (B=2, S=2048, H=12, DM=768, DH=64).

Sharding: 24 (batch, head) pairs -> 8 cores x 3 pairs. Host pre-transposes the
per-pair activations to [DM, S] in partition-major quarter-blocked layout
[P, NQ, NCH, GW] (one contiguous 3KB DMA line per partition per quarter);
xq/xk are fp8e3 (e3m4), xv fp16 (V-path quantization propagates ~1:1 to the
output; the QK path is dampened by softmax). Weights are fp16; matmuls mix
fp16 lhsT with fp8 rhs at full rate.

Per pair:
  Q^T/K^T/V^T computed per S-quarter as serial M=64 matmuls accumulating 6
  d_model chunks in a single PSUM bank (col-packed concurrent tiles only
  co-stream ~25% of the time -- LDWEIGHTS with a shared row group cannot be
  pulled ahead -- and the second bank is better spent on the outproj).
  K^T is evicted split by sk-block parity (even blocks -> partitions 0:64,
  odd -> 64:128) so score row-packing needs no K duplication; Q^T is evicted
  once and duplicated to partitions 64:128 by an SBUF->SBUF DMA (off the
  compute engines). scores^T pair = two K=64 matmuls row-packed at rows 0/64.
  P_u = exp(0.125 scores^T) on ACT; diagonal blocks masked in place on DVE.
  Z runs two score-pairs behind (lag-2) so the in-order PE queue never parks
  on an exp chain; Zaug (ones column -> denominators in row 64) accumulates in
  one bank. Denominator transposes are deferred into the next group's scores
  stream. Outproj accumulates [128, 2, 512] (two banks, bank-aligned mh
  slices) and evicts both halves in ONE op times 1/denom.

Scheduling: pair 0 emits DMAs + K/Q quarter-0 serially, then attention(p)
interleaves 1:1 with a background stream = [rest of pair p's projections,
pair p+1's projections], so attention starts as soon as the first quarter
lands. Z emission is gated on a vt_done counter (the Tile dep tracker only
sees already-emitted writers). Input DMAs issue from the otherwise-idle
GPSIMD queue so their ring-slot waits never block the output DMAs (Sync).
"""

import numpy as np
import ml_dtypes

B, S, H, DM, DH = 2, 2048, 12, 768, 64
P = 128
NCORES = 8
PPC = (B * H) // NCORES   # pairs per core = 3
NCH = DM // P             # 6 d_model chunks
NG = 4                    # sq groups
GW = S // NG              # 512
NSK = S // P              # 16 sk tiles
VW = DH + 1               # 65 (V augmented with ones column)
NQ = 4                    # S quarters (= NG)
MH = DM // 2              # outproj m-half = 384
NT = GW // P              # q tiles per group = 4

NP_W = np.float16
NP_X8 = ml_dtypes.float8_e3m4

_NC_CACHE = {}


def _build_bass(use_bias):
    import concourse.mybir as mybir
    import concourse.tile as tile
    from concourse import bacc
    from contextlib import ExitStack

    dt = mybir.dt
    f32 = dt.float32
    f16 = dt.float16
    f8 = dt.float8e3
    AF = mybir.ActivationFunctionType

    nc = bacc.Bacc("TRN2", target_bir_lowering=False, debug=False)

    # x layouts: [pair][partition][quarter][chunk][col] (3KB DMA lines)
    xq = nc.dram_tensor("xqT", [PPC, P, NQ, NCH, GW], f8, kind="ExternalInput").ap()
    xk = nc.dram_tensor("xkT", [PPC, P, NQ, NCH, GW], f8, kind="ExternalInput").ap()
    xv = nc.dram_tensor("xvT", [PPC, P, NQ, NCH, GW], f16, kind="ExternalInput").ap()
    # weights: [pair][partition][chunk][e] (p-major, single DMA line/partition)
    wq = nc.dram_tensor("wq", [PPC, P, NCH * DH], f16, kind="ExternalInput").ap()
    wk = nc.dram_tensor("wk", [PPC, P, NCH * DH], f16, kind="ExternalInput").ap()
    wv = nc.dram_tensor("wv", [PPC, P, NCH * DH], f16, kind="ExternalInput").ap()
    if use_bias:
        bq = nc.dram_tensor("bq", [PPC, 1, DH], f16, kind="ExternalInput").ap()
        bk = nc.dram_tensor("bk", [PPC, 1, DH], f16, kind="ExternalInput").ap()
        bv = nc.dram_tensor("bv", [PPC, 1, DH], f16, kind="ExternalInput").ap()
        onesr = nc.dram_tensor(
            "ones_row", [1, GW], f16, kind="ExternalInput").ap()
    wo = nc.dram_tensor("wo", [PPC, VW, DM], f16, kind="ExternalInput").ap()
    mk = nc.dram_tensor("masks", [P, NG * GW], f16, kind="ExternalInput").ap()
    onesc = nc.dram_tensor("ones_col", [P, NSK, 1], f16, kind="ExternalInput").ap()
    idin = nc.dram_tensor("ident64", [DH, DH], f16, kind="ExternalInput").ap()
    # out (TRANSPOSED, unnormalized): [pair][group][partition(m within
    # block)][mb*GW + q]; host divides by the denominators
    outT = nc.dram_tensor("outT", [PPC, NG, P, NCH * GW], f16,
                          kind="ExternalOutput").ap()
    dnm = nc.dram_tensor("denoms", [PPC, NG, 1, GW], f16,
                         kind="ExternalOutput").ap()

    with tile.TileContext(nc) as tc, ExitStack() as ctx:
        consts = ctx.enter_context(tc.tile_pool(name="consts", bufs=1))
        wpool = ctx.enter_context(tc.tile_pool(name="wpool", bufs=2))
        xin8 = ctx.enter_context(tc.tile_pool(name="xin8", bufs=2))
        xin16 = ctx.enter_context(tc.tile_pool(name="xin16", bufs=2))
        prj = ctx.enter_context(tc.tile_pool(name="prj", bufs=2))
        expp = ctx.enter_context(tc.tile_pool(name="expp", bufs=8))
        smal = ctx.enter_context(tc.tile_pool(name="smal", bufs=4))
        obuf = ctx.enter_context(tc.tile_pool(name="obuf", bufs=2))
        psA = ctx.enter_context(tc.tile_pool(name="psA", bufs=1, space="PSUM"))
        psB = ctx.enter_context(tc.tile_pool(name="psB", bufs=1, space="PSUM"))
        ps_s2 = ctx.enter_context(tc.tile_pool(name="ps_s2", bufs=2, space="PSUM"))
        ps_z = ctx.enter_context(tc.tile_pool(name="ps_z", bufs=1, space="PSUM"))
        ps_o = ctx.enter_context(tc.tile_pool(name="ps_o", bufs=1, space="PSUM"))

        masks = consts.tile([P, NG * GW], f16)
        # first quarter separately so the HAM warmup matmuls (which read it)
        # can start ~4us earlier
        nc.sync.dma_start(masks[:, 0:GW], mk[:, 0:GW])
        nc.sync.dma_start(masks[:, GW:], mk[:, GW:])
        ident = consts.tile([P, DH], f16)
        nc.sync.dma_start(ident[0:DH, :], idin)
        nc.sync.dma_start(ident[DH:P, :], idin)
        if use_bias:
            ones = consts.tile([1, GW], f16)
            nc.sync.dma_start(ones[:], onesr)

        # outproj work queue: [zaug, recip-slot (filled late), p, g, wo_sb]
        pending = []

        def flush_outproj(drain=False):
            # transposed outproj: W_O blocks stationary (reused, cheap
            # LDWEIGHTS), zaug moving; output is o^T = W_O^T z_un per m-block,
            # evicted as a plain cast -- normalization happens on the host
            zaug_, sums_, p_, g_, wo_sb_ = pending.pop(0)
            ob = obuf.tile([P, NCH * GW], f16, tag="ob")
            for mb in range(NCH):
                if drain and mb % 2 == 1:
                    # at drain time the scores banks are free; alternate
                    # into them so the next matmul never waits an evict
                    o_ps = ps_s2.tile([P, 2 * GW], f32, tag="s2",
                                      name="o_ps")
                else:
                    o_ps = ps_o.tile([P, GW], f32, tag="o", name="o_ps")
                nc.tensor.matmul(
                    o_ps[:, 0:GW],
                    lhsT=wo_sb_[:, mb * P:(mb + 1) * P],
                    rhs=zaug_[:],
                    start=True,
                    stop=True,
                )
                dst = ob[:, mb * GW:(mb + 1) * GW]
                if mb % 3 == 2:
                    nc.scalar.copy(dst, o_ps[:, 0:GW])
                else:
                    nc.vector.tensor_copy(dst, o_ps[:, 0:GW])
                yield
                yield
            # drain-time outputs issue on the Scalar queue (program order puts
            # them right after the final evictions; the congested Sync queue
            # would delay the kernel's last transfer)
            (nc.scalar if drain else nc.sync).dma_start(outT[p_, g_], ob[:])
            nc.gpsimd.dma_start(dnm[p_, g_], sums_[:])

        def gen_proj(p, out):
            """DMAs + projections for pair p. Yields "dma" once after DMA
            emission (prime point), "head" after K/Q quarter 0."""
            # head-critical DMAs first: wk + xk quarter 0, wq + xq quarter 0.
            # For pair 0 the q-side goes on the Sync queue so both quarters'
            # transfers issue in parallel (startup latency).
            qeng = nc.sync if p == 0 else nc.gpsimd
            wk_sb = wpool.tile([P, NCH * DH], f16, tag="wk")
            nc.gpsimd.dma_start(wk_sb[:], wk[p])
            xk_sb = xin8.tile([P, NQ * NCH * GW], f8, tag="xk")
            xk_v = xk_sb[:].rearrange("p (q c s) -> p q c s", q=NQ, c=NCH)
            nc.gpsimd.dma_start(xk_v[:, 0], xk[p, :, 0])
            wq_sb = wpool.tile([P, NCH * DH], f16, tag="wq")
            qeng.dma_start(wq_sb[:], wq[p])
            xq_sb = xin8.tile([P, NQ * NCH * GW], f8, tag="xq")
            xq_v = xq_sb[:].rearrange("p (q c s) -> p q c s", q=NQ, c=NCH)
            qeng.dma_start(xq_v[:, 0], xq[p, :, 0])
            wv_sb = wpool.tile([P, NCH * DH], f16, tag="wv")
            nc.gpsimd.dma_start(wv_sb[:], wv[p])
            wo_sb = wpool.tile([VW, DM], f16, tag="wo")
            nc.gpsimd.dma_start(wo_sb[:], wo[p])
            if use_bias:
                bq_sb = wpool.tile([1, DH], f16, tag="bq")
                nc.gpsimd.dma_start(bq_sb[:], bq[p])
                bk_sb = wpool.tile([1, DH], f16, tag="bk")
                nc.gpsimd.dma_start(bk_sb[:], bk[p])
                bv_sb = wpool.tile([1, DH], f16, tag="bv")
                nc.gpsimd.dma_start(bv_sb[:], bv[p])
            out["wo"] = wo_sb

            xv_sb = xin16.tile([P, NQ * NCH * GW], f16, tag="xv")
            xv_v = xv_sb[:].rearrange("p (q c s) -> p q c s", q=NQ, c=NCH)
            for q in range(1, NQ):
                nc.gpsimd.dma_start(xk_v[:, q], xk[p, :, q])
                nc.gpsimd.dma_start(xq_v[:, q], xq[p, :, q])
            for q in range(NQ):
                nc.gpsimd.dma_start(xv_v[:, q], xv[p, :, q])
            vaug = prj.tile([P, NSK * VW], f16, tag="vaug")
            nc.sync.dma_start(
                vaug[:].rearrange("p (i w) -> p i w", w=VW)[:, :, DH:VW], onesc
            )
            out["vaug"] = vaug
            out["vt_done"] = 0
            out["qk_done"] = 0
            qt = prj.tile([P, S], f16, tag="qt")
            kt = prj.tile([P, S], f16, tag="kt")
            yield "dma"

            def qk_quarter(q):
                # col-packed: Q in PE columns 0:63 (psA), K in 64:127 (psB) --
                # two open accumulation groups must use separate banks
                pa = psA.tile([P, GW], f32, tag="u", name="qk_psa")
                pb = psB.tile([P, GW], f32, tag="u", name="qk_psb")
                for c in range(NCH):
                    nc.tensor.matmul(
                        pa[0:DH, :],
                        lhsT=wq_sb[:, c * DH:(c + 1) * DH],
                        rhs=xq_v[:, q, c, :],
                        start=(c == 0),
                        stop=(c == NCH - 1) and not use_bias,
                        tile_position=(0, 0),
                    )
                    nc.tensor.matmul(
                        pb[DH:P, :],
                        lhsT=wk_sb[:, c * DH:(c + 1) * DH],
                        rhs=xk_v[:, q, c, :],
                        start=(c == 0),
                        stop=(c == NCH - 1) and not use_bias,
                        tile_position=(0, 64),
                    )
                    yield
                if use_bias:
                    nc.tensor.matmul(
                        pa[0:DH, :], lhsT=bq_sb[:], rhs=ones[:],
                        start=False, stop=True, tile_position=(0, 0))
                    nc.tensor.matmul(
                        pb[DH:P, :], lhsT=bk_sb[:], rhs=ones[:],
                        start=False, stop=True, tile_position=(0, 64))
                    yield
                qs = slice(q * GW, (q + 1) * GW)
                nc.vector.tensor_copy(qt[0:DH, qs], pa[0:DH, :])
                # duplicate to partitions 64:128 off-engine (SBUF->SBUF DMA)
                nc.gpsimd.dma_start(qt[DH:P, qs], qt[0:DH, qs])
                src = pb[DH:P, :].rearrange(
                    "p (b two c) -> p b two c", b=2, two=2)
                de = kt[0:DH, qs].rearrange(
                    "p (b two c) -> p b two c", b=2, two=2)
                do = kt[DH:P, qs].rearrange(
                    "p (b two c) -> p b two c", b=2, two=2)
                nc.scalar.copy(de[:, :, 0, :], src[:, :, 0, :])
                nc.vector.tensor_copy(do[:, :, 1, :], src[:, :, 1, :])
                yield
                yield

            yield from qk_quarter(0)
            out["qt"] = qt
            out["kt"] = kt
            out["qk_done"] = 1
            yield "head"
            for q in range(1, NQ):
                yield from qk_quarter(q)
                out["qk_done"] = q + 1

            # ---- V projection (col-packed S-quarter pairs) + transposes ----
            vt = prj.tile([DH, S], f16, tag="vt")
            for qp in range(2):
                qa, qb = 2 * qp, 2 * qp + 1
                pa = psA.tile([P, GW], f32, tag="u", name="v_psa")
                pb = psB.tile([P, GW], f32, tag="u", name="v_psb")
                for c in range(NCH):
                    nc.tensor.matmul(
                        pa[0:DH, :],
                        lhsT=wv_sb[:, c * DH:(c + 1) * DH],
                        rhs=xv_v[:, qa, c, :],
                        start=(c == 0),
                        stop=(c == NCH - 1) and not use_bias,
                        tile_position=(0, 0),
                    )
                    nc.tensor.matmul(
                        pb[DH:P, :],
                        lhsT=wv_sb[:, c * DH:(c + 1) * DH],
                        rhs=xv_v[:, qb, c, :],
                        start=(c == 0),
                        stop=(c == NCH - 1) and not use_bias,
                        tile_position=(0, 64),
                    )
                    yield
                if use_bias:
                    nc.tensor.matmul(
                        pa[0:DH, :], lhsT=bv_sb[:], rhs=ones[:],
                        start=False, stop=True, tile_position=(0, 0))
                    nc.tensor.matmul(
                        pb[DH:P, :], lhsT=bv_sb[:], rhs=ones[:],
                        start=False, stop=True, tile_position=(0, 64))
                    yield
                nc.vector.tensor_copy(vt[:, qa * GW:(qa + 1) * GW], pa[0:DH, :])
                nc.scalar.copy(vt[:, qb * GW:(qb + 1) * GW], pb[DH:P, :])
                yield
                # transpose the four ready sk blocks of each finished quarter
                for i in range(8 * qp, 8 * qp + 8):
                    pool = psA if i % 2 == 0 else psB
                    tp = pool.tile([P, DH], f16, tag="u", name="vtr_ps")
                    nc.tensor.transpose(
                        tp[:], vt[:, i * P:(i + 1) * P], ident[0:DH, :]
                    )
                    nc.vector.tensor_copy(vaug[:, i * VW:i * VW + DH], tp[:])
                    out["vt_done"] = i + 1
                    yield

        def gen_att(p, tiles):
            # the background stream emits this pair's projections; spin until
            # the QK tiles exist (each yield advances the background by one)
            while "qt" not in tiles:
                yield
            qt, kt, wo_sb = tiles["qt"], tiles["kt"], tiles["wo"]
            vaug = tiles["vaug"]

            for g in range(NG):
                # emission-order guard: scores of group g read qt quarter g
                # and kt quarters 0..g; their evictions must be emitted first
                while tiles["qk_done"] <= g:
                    yield
                gs = slice(g * GW, (g + 1) * GW)
                nsk = 4 * (g + 1)
                zctx = {"ps": None}

                def emit_scores_pair(ip, g=g, gs=gs):
                    s_ps = ps_s2.tile([P, 2 * GW], f32, tag="s2")
                    nc.tensor.matmul(
                        s_ps[:, 0:GW],
                        lhsT=kt[0:DH, ip * P:(ip + 1) * P],
                        rhs=qt[0:DH, gs],
                        start=True, stop=True,
                        tile_position=(0, 0),
                    )
                    nc.tensor.matmul(
                        s_ps[:, GW:2 * GW],
                        lhsT=kt[DH:P, (ip + 1) * P:(ip + 2) * P],
                        rhs=qt[DH:P, gs],
                        start=True, stop=True,
                        tile_position=(64, 0),
                    )
                    e_sb = expp.tile([P, 2 * GW], f16, tag="exp")
                    nc.scalar.activation(e_sb[:], s_ps[:], AF.Exp, scale=0.125)
                    if ip >= 4 * g:
                        j = ip - 4 * g
                        nc.vector.tensor_mul(
                            e_sb[:], e_sb[:], masks[:, j * GW:(j + 2) * GW])
                    return e_sb

                def emit_z(ip, e_use, nsk=nsk, zctx=zctx):
                    for k in range(2):
                        i = ip + k
                        # emission-order guard: the transpose writing vaug
                        # block i must be EMITTED before this read (the Tile
                        # dep tracker only sees already-emitted writers)
                        while tiles["vt_done"] <= i:
                            yield
                        if zctx["ps"] is None:
                            zctx["ps"] = ps_z.tile(
                                [VW, GW], f32, tag="z", name="z_ps")
                        nc.tensor.matmul(
                            zctx["ps"][:],
                            lhsT=vaug[:, i * VW:(i + 1) * VW],
                            rhs=e_use[:, k * GW:(k + 1) * GW],
                            start=(i == 0),
                            stop=(i == nsk - 1),
                        )
                        yield

                # z runs lag-2 behind scores
                eq = []
                for ip in range(0, nsk, 2):
                    eq.append((ip, emit_scores_pair(ip)))
                    yield
                    # eager flush: the previous group's outproj goes out
                    # right away (keeps the tail short)
                    if pending:
                        yield from flush_outproj()
                    if len(eq) > 2:
                        ip0, e0 = eq.pop(0)
                        yield from emit_z(ip0, e0)
                while eq:
                    ip0, e0 = eq.pop(0)
                    yield from emit_z(ip0, e0)

                z_ps = zctx["ps"]
                zaug = smal.tile([VW, GW], f16, tag="zaug")
                nc.scalar.copy(zaug[:], z_ps[:])
                sums0 = smal.tile([1, GW], f16, tag="sums0")
                nc.vector.tensor_copy(sums0[:], z_ps[DH:VW, :])
                pending.append([zaug, sums0, p, g, wo_sb])

        def interleave(a, b, bpulls=2):
            """Pull a once and b `bpulls` times per cycle until a exhausts;
            b is a shared background stream that survives across calls.
            Front-loading b keeps the PE instruction stream dense (HAM)."""
            a_live = True
            while a_live:
                try:
                    next(a)
                except StopIteration:
                    a_live = False
                for _ in range(bpulls):
                    if b is None:
                        break
                    try:
                        next(b)
                    except StopIteration:
                        b = None
            return b

        def chain(*gens):
            for g in gens:
                yield from g

        tiles = [{} for _ in range(PPC)]
        gens = [gen_proj(p, tiles[p]) for p in range(PPC)]
        # prime pair-0 DMAs, then warm the HAM clock gate with throwaway
        # matmuls on the masks tile (lands ~2us in) so the real projections
        # start at 2.4 GHz instead of ramping from 1.2 until ~36us
        next(gens[0])
        warm_ps = psA.tile([DH, GW], f32, tag="u", name="warm_ps")
        for r in range(10):
            nc.tensor.matmul(
                warm_ps[:],
                lhsT=masks[0:P, 0:DH],
                rhs=masks[0:P, 0:GW],
                start=(r == 0),
                stop=(r == 9),
            )
        # finish the pair-0 head (first K/Q quarter) serially
        for v in gens[0]:
            if v == "head":
                break
        # background: rest of proj(0), then proj(1), proj(2)
        bg = chain(*gens)
        for p in range(PPC):
            # front-load projections during pair 0 (dense PE warms the HAM
            # clock gate); 1:1 after so emitted proj matmuls never get far
            # enough ahead of their input DMAs to head-block the PE queue
            bg = interleave(gen_att(p, tiles[p]), bg, bpulls=1)
        while bg is not None:
            try:
                next(bg)
            except StopIteration:
                bg = None
        while pending:
            for _ in flush_outproj(drain=True):
                pass

    nc.compile()
    return nc


def get_nc(use_bias=False):
    if use_bias not in _NC_CACHE:
        _NC_CACHE[use_bias] = _build_bass(use_bias)
    return _NC_CACHE[use_bias]


def _pairs_for_core(c):
    return [(idx // H, idx % H) for idx in range(c * PPC, (c + 1) * PPC)]


def make_masks():
    # mask[p, (j c)] = 1.0 iff key pos 128*j + p <= query pos c (within block)
    j = np.arange(NG)[None, :, None]
    p = np.arange(P)[:, None, None]
    f = np.arange(GW)[None, None, :]
    return (f >= P * j + p).astype(NP_W).reshape(P, NG * GW)


def _xT_quarters(x, b, h, np_dt):
    # [S, DM] -> [DM, S] -> [P, NQ, NCH, GW] (partition-major quarter blocks)
    xt = x[b, :, h, :].T.astype(np_dt)          # [DM, S]
    xt = xt.reshape(NCH, P, NQ, GW)
    return np.ascontiguousarray(xt.transpose(1, 2, 0, 3))


def make_in_maps(inputs, use_bias):
    xq = np.asarray(inputs["normalized_resid_pre_q"], dtype=np.float32)
    xk = np.asarray(inputs["normalized_resid_pre_k"], dtype=np.float32)
    xv = np.asarray(inputs["normalized_resid_pre_v"], dtype=np.float32)
    W_Q = np.asarray(inputs["W_Q"], dtype=np.float32)
    W_K = np.asarray(inputs["W_K"], dtype=np.float32)
    W_V = np.asarray(inputs["W_V"], dtype=np.float32)
    b_Q = np.asarray(inputs["b_Q"], dtype=np.float32)
    b_K = np.asarray(inputs["b_K"], dtype=np.float32)
    b_V = np.asarray(inputs["b_V"], dtype=np.float32)
    W_O = np.asarray(inputs["W_O"], dtype=np.float32)
    b_O = np.asarray(inputs["b_O"], dtype=np.float32)

    def w_pmajor(W):
        # [DM, DH] -> [NCH, P, DH] -> [P, NCH*DH]
        w = W.astype(NP_W).reshape(NCH, P, DH)
        return np.ascontiguousarray(w.transpose(1, 0, 2)).reshape(P, NCH * DH)

    masks = make_masks()
    onesc = np.ones((P, NSK, 1), NP_W)
    ident64 = np.eye(DH, dtype=NP_W)
    in_maps = []
    for c in range(NCORES):
        pairs = _pairs_for_core(c)
        m = {
            "xqT": np.stack([_xT_quarters(xq, b, h, NP_X8) for b, h in pairs]),
            "xkT": np.stack([_xT_quarters(xk, b, h, NP_X8) for b, h in pairs]),
            "xvT": np.stack([_xT_quarters(xv, b, h, NP_W) for b, h in pairs]),
            "wq": np.stack([w_pmajor(W_Q[h]) for b, h in pairs]),
            "wk": np.stack([w_pmajor(W_K[h]) for b, h in pairs]),
            "wv": np.stack([w_pmajor(W_V[h]) for b, h in pairs]),
            "wo": np.stack(
                [np.concatenate([W_O[h], (b_O / H)[None, :]], axis=0).astype(NP_W)
                 for b, h in pairs]),
            "masks": masks,
            "ones_col": onesc,
            "ident64": ident64,
        }
        if use_bias:
            m["bq"] = np.stack([b_Q[h][None, :].astype(NP_W) for b, h in pairs])
            m["bk"] = np.stack([b_K[h][None, :].astype(NP_W) for b, h in pairs])
            m["bv"] = np.stack([b_V[h][None, :].astype(NP_W) for b, h in pairs])
            m["ones_row"] = np.ones((1, GW), NP_W)
        in_maps.append(m)
    return in_maps


def needs_bias(inputs):
    return any(
        np.any(np.asarray(inputs[k])) for k in ("b_Q", "b_K", "b_V")
    )


def assemble_output(results):
    out = np.empty((B, S, H, DM), np.float32)
    for c in range(NCORES):
        for j, (b, h) in enumerate(_pairs_for_core(c)):
            # outT[j]: [NG, P(m within block), NCH, GW] transposed
            # unnormalized; denoms[j]: [NG, 1, GW]
            o = results[c]["outT"][j].astype(np.float32)
            o = o.reshape(NG, P, NCH, GW).transpose(0, 3, 2, 1).reshape(S, DM)
            d = results[c]["denoms"][j].astype(np.float32).reshape(S, 1)
            out[b, :, h, :] = o / d
    return out


def kernel(**inputs):
    from concourse import bass_utils

    use_bias = needs_bias(inputs)
    nc = get_nc(use_bias)
    in_maps = make_in_maps(inputs, use_bias)
    res = bass_utils.run_bass_kernel_spmd(nc, in_maps, core_ids=list(range(NCORES)))
    return assemble_output(res.results)


# revision 46
# speedup vs baseline: 1.0290x; 1.0290x over previous
"""Bass/Trainium2 kernel for per-head attention (B=2, S=2048, H=12, DM=768, DH=64).

Sharding: 24 (batch, head) pairs -> 8 cores x 3 pairs. Host pre-transposes the
per-pair activations to [DM, S] in partition-major quarter-blocked layout
[P, NQ, NCH, GW] (one contiguous 3KB DMA line per partition per quarter);
xq/xk are fp8e3 (e3m4), xv fp16 (V-path quantization propagates ~1:1 to the
output; the QK path is dampened by softmax). Weights are fp16; matmuls mix
fp16 lhsT with fp8 rhs at full rate.

Per pair:
  Q^T/K^T/V^T computed per S-quarter as serial M=64 matmuls accumulating 6
  d_model chunks in a single PSUM bank (col-packed concurrent tiles only
  co-stream ~25% of the time -- LDWEIGHTS with a shared row group cannot be
  pulled ahead -- and the second bank is better spent on the outproj).
  K^T is evicted split by sk-block parity (even blocks -> partitions 0:64,
  odd -> 64:128) so score row-packing needs no K duplication; Q^T is evicted
  once and duplicated to partitions 64:128 by an SBUF->SBUF DMA (off the
  compute engines). scores^T pair = two K=64 matmuls row-packed at rows 0/64.
  P_u = exp(0.125 scores^T) on ACT; diagonal blocks masked in place on DVE.
  Z runs two score-pairs behind (lag-2) so the in-order PE queue never parks
  on an exp chain; Zaug (ones column -> denominators in row 64) accumulates in
  one bank. Denominator transposes are deferred into the next group's scores
  stream. Outproj accumulates [128, 2, 512] (two banks, bank-aligned mh
  slices) and evicts both halves in ONE op times 1/denom.

Scheduling: pair 0 emits DMAs + K/Q quarter-0 serially, then attention(p)
interleaves 1:1 with a background stream = [rest of pair p's projections,
pair p+1's projections], so attention starts as soon as the first quarter
lands. Z emission is gated on a vt_done counter (the Tile dep tracker only
sees already-emitted writers). Input DMAs issue from the otherwise-idle
GPSIMD queue so their ring-slot waits never block the output DMAs (Sync).
"""

import numpy as np
import ml_dtypes

B, S, H, DM, DH = 2, 2048, 12, 768, 64
P = 128
NCORES = 8
PPC = (B * H) // NCORES   # pairs per core = 3
NCH = DM // P             # 6 d_model chunks
NG = 4                    # sq groups
GW = S // NG              # 512
NSK = S // P              # 16 sk tiles
VW = DH + 1               # 65 (V augmented with ones column)
NQ = 4                    # S quarters (= NG)
MH = DM // 2              # outproj m-half = 384
NT = GW // P              # q tiles per group = 4

NP_W = np.float16
NP_X8 = ml_dtypes.float8_e3m4

_NC_CACHE = {}


def _build_bass(use_bias):
    import concourse.mybir as mybir
    import concourse.tile as tile
    from concourse import bacc
    from contextlib import ExitStack

    dt = mybir.dt
    f32 = dt.float32
    f16 = dt.float16
    f8 = dt.float8e3
    AF = mybir.ActivationFunctionType

    nc = bacc.Bacc("TRN2", target_bir_lowering=False, debug=False)

    # x layouts: [pair][partition][quarter][chunk][col] (3KB DMA lines)
    xq = nc.dram_tensor("xqT", [PPC, P, NQ, NCH, GW], f8, kind="ExternalInput").ap()
    xk = nc.dram_tensor("xkT", [PPC, P, NQ, NCH, GW], f8, kind="ExternalInput").ap()
    xv = nc.dram_tensor("xvT", [PPC, P, NQ, NCH, GW], f16, kind="ExternalInput").ap()
    # weights: [pair][partition][chunk][e] (p-major, single DMA line/partition)
    wq = nc.dram_tensor("wq", [PPC, P, NCH * DH], f16, kind="ExternalInput").ap()
    wk = nc.dram_tensor("wk", [PPC, P, NCH * DH], f16, kind="ExternalInput").ap()
    wv = nc.dram_tensor("wv", [PPC, P, NCH * DH], f16, kind="ExternalInput").ap()
    if use_bias:
        bq = nc.dram_tensor("bq", [PPC, 1, DH], f16, kind="ExternalInput").ap()
        bk = nc.dram_tensor("bk", [PPC, 1, DH], f16, kind="ExternalInput").ap()
        bv = nc.dram_tensor("bv", [PPC, 1, DH], f16, kind="ExternalInput").ap()
        onesr = nc.dram_tensor(
            "ones_row", [1, GW], f16, kind="ExternalInput").ap()
    wo = nc.dram_tensor("wo", [PPC, VW, DM], f16, kind="ExternalInput").ap()
    mk = nc.dram_tensor("masks", [P, NG * GW], f16, kind="ExternalInput").ap()
    onesc = nc.dram_tensor("ones_col", [P, NSK, 1], f16, kind="ExternalInput").ap()
    idin = nc.dram_tensor("ident64", [DH, DH], f16, kind="ExternalInput").ap()
    # out (TRANSPOSED, unnormalized): [pair][group][partition(m within
    # block)][mb*GW + q]; host divides by the denominators
    outT = nc.dram_tensor("outT", [PPC, NG, P, NCH * GW], f16,
                          kind="ExternalOutput").ap()
    dnm = nc.dram_tensor("denoms", [PPC, NG, 1, GW], f16,
                         kind="ExternalOutput").ap()

    with tile.TileContext(nc) as tc, ExitStack() as ctx:
        consts = ctx.enter_context(tc.tile_pool(name="consts", bufs=1))
        wpool = ctx.enter_context(tc.tile_pool(name="wpool", bufs=2))
        xin8 = ctx.enter_context(tc.tile_pool(name="xin8", bufs=2))
        xin16 = ctx.enter_context(tc.tile_pool(name="xin16", bufs=2))
        prj = ctx.enter_context(tc.tile_pool(name="prj", bufs=2))
        expp = ctx.enter_context(tc.tile_pool(name="expp", bufs=8))
        smal = ctx.enter_context(tc.tile_pool(name="smal", bufs=4))
        obuf = ctx.enter_context(tc.tile_pool(name="obuf", bufs=2))
        psA = ctx.enter_context(tc.tile_pool(name="psA", bufs=1, space="PSUM"))
        psB = ctx.enter_context(tc.tile_pool(name="psB", bufs=1, space="PSUM"))
        ps_s2 = ctx.enter_context(tc.tile_pool(name="ps_s2", bufs=2, space="PSUM"))
        ps_z = ctx.enter_context(tc.tile_pool(name="ps_z", bufs=1, space="PSUM"))
        ps_o = ctx.enter_context(tc.tile_pool(name="ps_o", bufs=1, space="PSUM"))

        masks = consts.tile([P, NG * GW], f16)
        nc.sync.dma_start(masks[:], mk)
        ident = consts.tile([P, DH], f16)
        nc.sync.dma_start(ident[0:DH, :], idin)
        nc.sync.dma_start(ident[DH:P, :], idin)
        if use_bias:
            ones = consts.tile([1, GW], f16)
            nc.sync.dma_start(ones[:], onesr)

        # outproj work queue: [zaug, recip-slot (filled late), p, g, wo_sb]
        pending = []

        def flush_outproj(drain=False):
            # transposed outproj: W_O blocks stationary (reused, cheap
            # LDWEIGHTS), zaug moving; output is o^T = W_O^T z_un per m-block,
            # evicted as a plain cast -- normalization happens on the host
            zaug_, sums_, p_, g_, wo_sb_ = pending.pop(0)
            ob = obuf.tile([P, NCH * GW], f16, tag="ob")
            for mb in range(NCH):
                if drain and mb % 2 == 1:
                    # at drain time the scores banks are free; alternate
                    # into them so the next matmul never waits an evict
                    o_ps = ps_s2.tile([P, 2 * GW], f32, tag="s2",
                                      name="o_ps")
                else:
                    o_ps = ps_o.tile([P, GW], f32, tag="o", name="o_ps")
                nc.tensor.matmul(
                    o_ps[:, 0:GW],
                    lhsT=wo_sb_[:, mb * P:(mb + 1) * P],
                    rhs=zaug_[:],
                    start=True,
                    stop=True,
                )
                dst = ob[:, mb * GW:(mb + 1) * GW]
                if mb % 3 == 2:
                    nc.scalar.copy(dst, o_ps[:, 0:GW])
                else:
                    nc.vector.tensor_copy(dst, o_ps[:, 0:GW])
                yield
                yield
            # drain-time outputs issue on the Scalar queue (program order puts
            # them right after the final evictions; the congested Sync queue
            # would delay the kernel's last transfer)
            (nc.scalar if drain else nc.sync).dma_start(outT[p_, g_], ob[:])
            nc.gpsimd.dma_start(dnm[p_, g_], sums_[:])

        def gen_proj(p, out):
            """DMAs + projections for pair p. Yields "dma" once after DMA
            emission (prime point), "head" after K/Q quarter 0."""
            # head-critical DMAs first: wk + xk quarter 0, wq + xq quarter 0.
            # For pair 0 the q-side goes on the Sync queue so both quarters'
            # transfers issue in parallel (startup latency).
            qeng = nc.sync if p == 0 else nc.gpsimd
            wk_sb = wpool.tile([P, NCH * DH], f16, tag="wk")
            nc.gpsimd.dma_start(wk_sb[:], wk[p])
            xk_sb = xin8.tile([P, NQ * NCH * GW], f8, tag="xk")
            xk_v = xk_sb[:].rearrange("p (q c s) -> p q c s", q=NQ, c=NCH)
            nc.gpsimd.dma_start(xk_v[:, 0], xk[p, :, 0])
            wq_sb = wpool.tile([P, NCH * DH], f16, tag="wq")
            qeng.dma_start(wq_sb[:], wq[p])
            xq_sb = xin8.tile([P, NQ * NCH * GW], f8, tag="xq")
            xq_v = xq_sb[:].rearrange("p (q c s) -> p q c s", q=NQ, c=NCH)
            qeng.dma_start(xq_v[:, 0], xq[p, :, 0])
            wv_sb = wpool.tile([P, NCH * DH], f16, tag="wv")
            nc.gpsimd.dma_start(wv_sb[:], wv[p])
            wo_sb = wpool.tile([VW, DM], f16, tag="wo")
            nc.gpsimd.dma_start(wo_sb[:], wo[p])
            if use_bias:
                bq_sb = wpool.tile([1, DH], f16, tag="bq")
                nc.gpsimd.dma_start(bq_sb[:], bq[p])
                bk_sb = wpool.tile([1, DH], f16, tag="bk")
                nc.gpsimd.dma_start(bk_sb[:], bk[p])
                bv_sb = wpool.tile([1, DH], f16, tag="bv")
                nc.gpsimd.dma_start(bv_sb[:], bv[p])
            out["wo"] = wo_sb

            xv_sb = xin16.tile([P, NQ * NCH * GW], f16, tag="xv")
            xv_v = xv_sb[:].rearrange("p (q c s) -> p q c s", q=NQ, c=NCH)
            for q in range(1, NQ):
                nc.gpsimd.dma_start(xk_v[:, q], xk[p, :, q])
                nc.gpsimd.dma_start(xq_v[:, q], xq[p, :, q])
            for q in range(NQ):
                nc.gpsimd.dma_start(xv_v[:, q], xv[p, :, q])
            vaug = prj.tile([P, NSK * VW], f16, tag="vaug")
            nc.sync.dma_start(
                vaug[:].rearrange("p (i w) -> p i w", w=VW)[:, :, DH:VW], onesc
            )
            out["vaug"] = vaug
            out["vt_done"] = 0
            out["qk_done"] = 0
            qt = prj.tile([P, S], f16, tag="qt")
            kt = prj.tile([P, S], f16, tag="kt")
            yield "dma"

            def qk_quarter(q):
                # col-packed: Q in PE columns 0:63 (psA), K in 64:127 (psB) --
                # two open accumulation groups must use separate banks
                pa = psA.tile([P, GW], f32, tag="u", name="qk_psa")
                pb = psB.tile([P, GW], f32, tag="u", name="qk_psb")
                for c in range(NCH):
                    nc.tensor.matmul(
                        pa[0:DH, :],
                        lhsT=wq_sb[:, c * DH:(c + 1) * DH],
                        rhs=xq_v[:, q, c, :],
                        start=(c == 0),
                        stop=(c == NCH - 1) and not use_bias,
                        tile_position=(0, 0),
                    )
                    nc.tensor.matmul(
                        pb[DH:P, :],
                        lhsT=wk_sb[:, c * DH:(c + 1) * DH],
                        rhs=xk_v[:, q, c, :],
                        start=(c == 0),
                        stop=(c == NCH - 1) and not use_bias,
                        tile_position=(0, 64),
                    )
                    yield
                if use_bias:
                    nc.tensor.matmul(
                        pa[0:DH, :], lhsT=bq_sb[:], rhs=ones[:],
                        start=False, stop=True, tile_position=(0, 0))
                    nc.tensor.matmul(
                        pb[DH:P, :], lhsT=bk_sb[:], rhs=ones[:],
                        start=False, stop=True, tile_position=(0, 64))
                    yield
                qs = slice(q * GW, (q + 1) * GW)
                nc.vector.tensor_copy(qt[0:DH, qs], pa[0:DH, :])
                # duplicate to partitions 64:128 off-engine (SBUF->SBUF DMA)
                nc.gpsimd.dma_start(qt[DH:P, qs], qt[0:DH, qs])
                src = pb[DH:P, :].rearrange(
                    "p (b two c) -> p b two c", b=2, two=2)
                de = kt[0:DH, qs].rearrange(
                    "p (b two c) -> p b two c", b=2, two=2)
                do = kt[DH:P, qs].rearrange(
                    "p (b two c) -> p b two c", b=2, two=2)
                nc.scalar.copy(de[:, :, 0, :], src[:, :, 0, :])
                nc.vector.tensor_copy(do[:, :, 1, :], src[:, :, 1, :])
                yield
                yield

            yield from qk_quarter(0)
            out["qt"] = qt
            out["kt"] = kt
            out["qk_done"] = 1
            yield "head"
            for q in range(1, NQ):
                yield from qk_quarter(q)
                out["qk_done"] = q + 1

            # ---- V projection (col-packed S-quarter pairs) + transposes ----
            vt = prj.tile([DH, S], f16, tag="vt")
            for qp in range(2):
                qa, qb = 2 * qp, 2 * qp + 1
                pa = psA.tile([P, GW], f32, tag="u", name="v_psa")
                pb = psB.tile([P, GW], f32, tag="u", name="v_psb")
                for c in range(NCH):
                    nc.tensor.matmul(
                        pa[0:DH, :],
                        lhsT=wv_sb[:, c * DH:(c + 1) * DH],
                        rhs=xv_v[:, qa, c, :],
                        start=(c == 0),
                        stop=(c == NCH - 1) and not use_bias,
                        tile_position=(0, 0),
                    )
                    nc.tensor.matmul(
                        pb[DH:P, :],
                        lhsT=wv_sb[:, c * DH:(c + 1) * DH],
                        rhs=xv_v[:, qb, c, :],
                        start=(c == 0),
                        stop=(c == NCH - 1) and not use_bias,
                        tile_position=(0, 64),
                    )
                    yield
                if use_bias:
                    nc.tensor.matmul(
                        pa[0:DH, :], lhsT=bv_sb[:], rhs=ones[:],
                        start=False, stop=True, tile_position=(0, 0))
                    nc.tensor.matmul(
                        pb[DH:P, :], lhsT=bv_sb[:], rhs=ones[:],
                        start=False, stop=True, tile_position=(0, 64))
                    yield
                nc.vector.tensor_copy(vt[:, qa * GW:(qa + 1) * GW], pa[0:DH, :])
                nc.scalar.copy(vt[:, qb * GW:(qb + 1) * GW], pb[DH:P, :])
                yield
                # transpose the four ready sk blocks of each finished quarter
                for i in range(8 * qp, 8 * qp + 8):
                    pool = psA if i % 2 == 0 else psB
                    tp = pool.tile([P, DH], f16, tag="u", name="vtr_ps")
                    nc.tensor.transpose(
                        tp[:], vt[:, i * P:(i + 1) * P], ident[0:DH, :]
                    )
                    nc.vector.tensor_copy(vaug[:, i * VW:i * VW + DH], tp[:])
                    out["vt_done"] = i + 1
                    yield

        def gen_att(p, tiles):
            # the background stream emits this pair's projections; spin until
            # the QK tiles exist (each yield advances the background by one)
            while "qt" not in tiles:
                yield
            qt, kt, wo_sb = tiles["qt"], tiles["kt"], tiles["wo"]
            vaug = tiles["vaug"]

            for g in range(NG):
                # emission-order guard: scores of group g read qt quarter g
                # and kt quarters 0..g; their evictions must be emitted first
                while tiles["qk_done"] <= g:
                    yield
                gs = slice(g * GW, (g + 1) * GW)
                nsk = 4 * (g + 1)
                zctx = {"ps": None}

                def emit_scores_pair(ip, g=g, gs=gs):
                    s_ps = ps_s2.tile([P, 2 * GW], f32, tag="s2")
                    nc.tensor.matmul(
                        s_ps[:, 0:GW],
                        lhsT=kt[0:DH, ip * P:(ip + 1) * P],
                        rhs=qt[0:DH, gs],
                        start=True, stop=True,
                        tile_position=(0, 0),
                    )
                    nc.tensor.matmul(
                        s_ps[:, GW:2 * GW],
                        lhsT=kt[DH:P, (ip + 1) * P:(ip + 2) * P],
                        rhs=qt[DH:P, gs],
                        start=True, stop=True,
                        tile_position=(64, 0),
                    )
                    e_sb = expp.tile([P, 2 * GW], f16, tag="exp")
                    nc.scalar.activation(e_sb[:], s_ps[:], AF.Exp, scale=0.125)
                    if ip >= 4 * g:
                        j = ip - 4 * g
                        nc.vector.tensor_mul(
                            e_sb[:], e_sb[:], masks[:, j * GW:(j + 2) * GW])
                    return e_sb

                def emit_z(ip, e_use, nsk=nsk, zctx=zctx):
                    for k in range(2):
                        i = ip + k
                        # emission-order guard: the transpose writing vaug
                        # block i must be EMITTED before this read (the Tile
                        # dep tracker only sees already-emitted writers)
                        while tiles["vt_done"] <= i:
                            yield
                        if zctx["ps"] is None:
                            zctx["ps"] = ps_z.tile(
                                [VW, GW], f32, tag="z", name="z_ps")
                        nc.tensor.matmul(
                            zctx["ps"][:],
                            lhsT=vaug[:, i * VW:(i + 1) * VW],
                            rhs=e_use[:, k * GW:(k + 1) * GW],
                            start=(i == 0),
                            stop=(i == nsk - 1),
                        )
                        yield

                # z runs lag-2 behind scores
                eq = []
                for ip in range(0, nsk, 2):
                    eq.append((ip, emit_scores_pair(ip)))
                    yield
                    # eager flush: the previous group's outproj goes out
                    # right away (keeps the tail short)
                    if pending:
                        yield from flush_outproj()
                    if len(eq) > 2:
                        ip0, e0 = eq.pop(0)
                        yield from emit_z(ip0, e0)
                while eq:
                    ip0, e0 = eq.pop(0)
                    yield from emit_z(ip0, e0)

                z_ps = zctx["ps"]
                zaug = smal.tile([VW, GW], f16, tag="zaug")
                nc.scalar.copy(zaug[:], z_ps[:])
                sums0 = smal.tile([1, GW], f16, tag="sums0")
                nc.vector.tensor_copy(sums0[:], z_ps[DH:VW, :])
                pending.append([zaug, sums0, p, g, wo_sb])

        def interleave(a, b, bpulls=2):
            """Pull a once and b `bpulls` times per cycle until a exhausts;
            b is a shared background stream that survives across calls.
            Front-loading b keeps the PE instruction stream dense (HAM)."""
            a_live = True
            while a_live:
                try:
                    next(a)
                except StopIteration:
                    a_live = False
                for _ in range(bpulls):
                    if b is None:
                        break
                    try:
                        next(b)
                    except StopIteration:
                        b = None
            return b

        def chain(*gens):
            for g in gens:
                yield from g

        tiles = [{} for _ in range(PPC)]
        gens = [gen_proj(p, tiles[p]) for p in range(PPC)]
        # prime pair-0 DMAs, then warm the HAM clock gate with throwaway
        # matmuls on the masks tile (lands ~2us in) so the real projections
        # start at 2.4 GHz instead of ramping from 1.2 until ~36us
        next(gens[0])
        warm_ps = psA.tile([DH, GW], f32, tag="u", name="warm_ps")
        for r in range(10):
            nc.tensor.matmul(
                warm_ps[:],
                lhsT=masks[0:P, 0:DH],
                rhs=masks[0:P, GW:2 * GW],
                start=(r == 0),
                stop=(r == 9),
            )
        # finish the pair-0 head (first K/Q quarter) serially
        for v in gens[0]:
            if v == "head":
                break
        # background: rest of proj(0), then proj(1), proj(2)
        bg = chain(*gens)
        for p in range(PPC):
            # front-load projections during pair 0 (dense PE warms the HAM
            # clock gate); 1:1 after so emitted proj matmuls never get far
            # enough ahead of their input DMAs to head-block the PE queue
            bg = interleave(gen_att(p, tiles[p]), bg, bpulls=1)
        while bg is not None:
            try:
                next(bg)
            except StopIteration:
                bg = None
        while pending:
            for _ in flush_outproj(drain=True):
                pass

    nc.compile()
    return nc


def get_nc(use_bias=False):
    if use_bias not in _NC_CACHE:
        _NC_CACHE[use_bias] = _build_bass(use_bias)
    return _NC_CACHE[use_bias]


def _pairs_for_core(c):
    return [(idx // H, idx % H) for idx in range(c * PPC, (c + 1) * PPC)]


def make_masks():
    # mask[p, (j c)] = 1.0 iff key pos 128*j + p <= query pos c (within block)
    j = np.arange(NG)[None, :, None]
    p = np.arange(P)[:, None, None]
    f = np.arange(GW)[None, None, :]
    return (f >= P * j + p).astype(NP_W).reshape(P, NG * GW)


def _xT_quarters(x, b, h, np_dt):
    # [S, DM] -> [DM, S] -> [P, NQ, NCH, GW] (partition-major quarter blocks)
    xt = x[b, :, h, :].T.astype(np_dt)          # [DM, S]
    xt = xt.reshape(NCH, P, NQ, GW)
    return np.ascontiguousarray(xt.transpose(1, 2, 0, 3))


def make_in_maps(inputs, use_bias):
    xq = np.asarray(inputs["normalized_resid_pre_q"], dtype=np.float32)
    xk = np.asarray(inputs["normalized_resid_pre_k"], dtype=np.float32)
    xv = np.asarray(inputs["normalized_resid_pre_v"], dtype=np.float32)
    W_Q = np.asarray(inputs["W_Q"], dtype=np.float32)
    W_K = np.asarray(inputs["W_K"], dtype=np.float32)
    W_V = np.asarray(inputs["W_V"], dtype=np.float32)
    b_Q = np.asarray(inputs["b_Q"], dtype=np.float32)
    b_K = np.asarray(inputs["b_K"], dtype=np.float32)
    b_V = np.asarray(inputs["b_V"], dtype=np.float32)
    W_O = np.asarray(inputs["W_O"], dtype=np.float32)
    b_O = np.asarray(inputs["b_O"], dtype=np.float32)

    def w_pmajor(W):
        # [DM, DH] -> [NCH, P, DH] -> [P, NCH*DH]
        w = W.astype(NP_W).reshape(NCH, P, DH)
        return np.ascontiguousarray(w.transpose(1, 0, 2)).reshape(P, NCH * DH)

    masks = make_masks()
    onesc = np.ones((P, NSK, 1), NP_W)
    ident64 = np.eye(DH, dtype=NP_W)
    in_maps = []
    for c in range(NCORES):
        pairs = _pairs_for_core(c)
        m = {
            "xqT": np.stack([_xT_quarters(xq, b, h, NP_X8) for b, h in pairs]),
            "xkT": np.stack([_xT_quarters(xk, b, h, NP_X8) for b, h in pairs]),
            "xvT": np.stack([_xT_quarters(xv, b, h, NP_W) for b, h in pairs]),
            "wq": np.stack([w_pmajor(W_Q[h]) for b, h in pairs]),
            "wk": np.stack([w_pmajor(W_K[h]) for b, h in pairs]),
            "wv": np.stack([w_pmajor(W_V[h]) for b, h in pairs]),
            "wo": np.stack(
                [np.concatenate([W_O[h], (b_O / H)[None, :]], axis=0).astype(NP_W)
                 for b, h in pairs]),
            "masks": masks,
            "ones_col": onesc,
            "ident64": ident64,
        }
        if use_bias:
            m["bq"] = np.stack([b_Q[h][None, :].astype(NP_W) for b, h in pairs])
            m["bk"] = np.stack([b_K[h][None, :].astype(NP_W) for b, h in pairs])
            m["bv"] = np.stack([b_V[h][None, :].astype(NP_W) for b, h in pairs])
            m["ones_row"] = np.ones((1, GW), NP_W)
        in_maps.append(m)
    return in_maps


def needs_bias(inputs):
    return any(
        np.any(np.asarray(inputs[k])) for k in ("b_Q", "b_K", "b_V")
    )


def assemble_output(results):
    out = np.empty((B, S, H, DM), np.float32)
    for c in range(NCORES):
        for j, (b, h) in enumerate(_pairs_for_core(c)):
            # outT[j]: [NG, P(m within block), NCH, GW] transposed
            # unnormalized; denoms[j]: [NG, 1, GW]
            o = results[c]["outT"][j].astype(np.float32)
            o = o.reshape(NG, P, NCH, GW).transpose(0, 3, 2, 1).reshape(S, DM)
            d = results[c]["denoms"][j].astype(np.float32).reshape(S, 1)
            out[b, :, h, :] = o / d
    return out


def kernel(**inputs):
    from concourse import bass_utils

    use_bias = needs_bias(inputs)
    nc = get_nc(use_bias)
    in_maps = make_in_maps(inputs, use_bias)
    res = bass_utils.run_bass_kernel_spmd(nc, in_maps, core_ids=list(range(NCORES)))
    return assemble_output(res.results)


# revision 50
# speedup vs baseline: 1.0303x; 1.0012x over previous
"""Bass/Trainium2 kernel for per-head attention (B=2, S=2048, H=12, DM=768, DH=64).

Sharding: 24 (batch, head) pairs -> 8 cores x 3 pairs. Host pre-transposes the
per-pair activations to [DM, S] in partition-major quarter-blocked layout
[P, NQ, NCH, GW] (one contiguous 3KB DMA line per partition per quarter);
xq/xk are fp8e3 (e3m4), xv fp16 (V-path quantization propagates ~1:1 to the
output; the QK path is dampened by softmax). Weights are fp16; matmuls mix
fp16 lhsT with fp8 rhs at full rate.

Per pair:
  Q^T/K^T/V^T computed per S-quarter as serial M=64 matmuls accumulating 6
  d_model chunks in a single PSUM bank (col-packed concurrent tiles only
  co-stream ~25% of the time -- LDWEIGHTS with a shared row group cannot be
  pulled ahead -- and the second bank is better spent on the outproj).
  K^T is evicted split by sk-block parity (even blocks -> partitions 0:64,
  odd -> 64:128) so score row-packing needs no K duplication; Q^T is evicted
  once and duplicated to partitions 64:128 by an SBUF->SBUF DMA (off the
  compute engines). scores^T pair = two K=64 matmuls row-packed at rows 0/64.
  P_u = exp(0.125 scores^T) on ACT; diagonal blocks masked in place on DVE.
  Z runs two score-pairs behind (lag-2) so the in-order PE queue never parks
  on an exp chain; Zaug (ones column -> denominators in row 64) accumulates in
  one bank. The output projection is TRANSPOSED: W_O m-blocks are the
  stationary operand (64-col LDWEIGHTS instead of reloading a 128-col zaug
  block per matmul) and zaug the moving one, producing unnormalized
  o^T = W_O^T z_un evicted as plain casts. The denominator row ships to the
  host, which divides (exact same math; the ones-row times b_O/H keeps the
  bias exact).

Scheduling: pair 0 emits DMAs + HAM-warmup matmuls + K/Q quarter-0 serially,
then attention(p) interleaves 1:1 with a background stream = [rest of pair
p's projections, pair p+1's projections], so attention starts as soon as the
first quarter lands. Z emission is gated on a vt_done counter and scores on a
qk_done counter (the Tile dep tracker only sees already-emitted writers).
Input DMAs issue from the otherwise-idle GPSIMD queue so their ring-slot
waits never block the output DMAs (Sync; Scalar for the drain tail).
"""

import numpy as np
import ml_dtypes

B, S, H, DM, DH = 2, 2048, 12, 768, 64
P = 128
NCORES = 8
PPC = (B * H) // NCORES   # pairs per core = 3
NCH = DM // P             # 6 d_model chunks
NG = 4                    # sq groups
GW = S // NG              # 512
NSK = S // P              # 16 sk tiles
VW = DH + 1               # 65 (V augmented with ones column)
NQ = 4                    # S quarters (= NG)
MH = DM // 2              # outproj m-half = 384
NT = GW // P              # q tiles per group = 4

NP_W = np.float16
NP_X8 = ml_dtypes.float8_e3m4

_NC_CACHE = {}


def _build_bass(use_bias):
    import concourse.mybir as mybir
    import concourse.tile as tile
    from concourse import bacc
    from contextlib import ExitStack

    dt = mybir.dt
    f32 = dt.float32
    f16 = dt.float16
    f8 = dt.float8e3
    AF = mybir.ActivationFunctionType

    nc = bacc.Bacc("TRN2", target_bir_lowering=False, debug=False)

    # x layouts: [pair][partition][quarter][chunk][col] (3KB DMA lines)
    xq = nc.dram_tensor("xqT", [PPC, P, NQ, NCH, GW], f8, kind="ExternalInput").ap()
    xk = nc.dram_tensor("xkT", [PPC, P, NQ, NCH, GW], f8, kind="ExternalInput").ap()
    xv = nc.dram_tensor("xvT", [PPC, P, NQ, NCH, GW], f16, kind="ExternalInput").ap()
    # weights: [pair][partition][chunk][e] (p-major, single DMA line/partition)
    wq = nc.dram_tensor("wq", [PPC, P, NCH * DH], f16, kind="ExternalInput").ap()
    wk = nc.dram_tensor("wk", [PPC, P, NCH * DH], f16, kind="ExternalInput").ap()
    wv = nc.dram_tensor("wv", [PPC, P, NCH * DH], f16, kind="ExternalInput").ap()
    if use_bias:
        bq = nc.dram_tensor("bq", [PPC, 1, DH], f16, kind="ExternalInput").ap()
        bk = nc.dram_tensor("bk", [PPC, 1, DH], f16, kind="ExternalInput").ap()
        bv = nc.dram_tensor("bv", [PPC, 1, DH], f16, kind="ExternalInput").ap()
        onesr = nc.dram_tensor(
            "ones_row", [1, GW], f16, kind="ExternalInput").ap()
    wo = nc.dram_tensor("wo", [PPC, VW, DM], f16, kind="ExternalInput").ap()
    mk = nc.dram_tensor("masks", [P, NG * GW], f16, kind="ExternalInput").ap()
    onesc = nc.dram_tensor("ones_col", [P, NSK, 1], f16, kind="ExternalInput").ap()
    idin = nc.dram_tensor("ident64", [DH, DH], f16, kind="ExternalInput").ap()
    # out (TRANSPOSED, unnormalized): [pair][group][partition(m within
    # block)][mb*GW + q]; host divides by the denominators
    outT = nc.dram_tensor("outT", [PPC, NG, P, NCH * GW], f16,
                          kind="ExternalOutput").ap()
    dnm = nc.dram_tensor("denoms", [PPC, NG, 1, GW], f16,
                         kind="ExternalOutput").ap()

    with tile.TileContext(nc) as tc, ExitStack() as ctx:
        consts = ctx.enter_context(tc.tile_pool(name="consts", bufs=1))
        wpool = ctx.enter_context(tc.tile_pool(name="wpool", bufs=2))
        xin8 = ctx.enter_context(tc.tile_pool(name="xin8", bufs=2))
        xin16 = ctx.enter_context(tc.tile_pool(name="xin16", bufs=2))
        prj = ctx.enter_context(tc.tile_pool(name="prj", bufs=2))
        expp = ctx.enter_context(tc.tile_pool(name="expp", bufs=8))
        smal = ctx.enter_context(tc.tile_pool(name="smal", bufs=4))
        obuf = ctx.enter_context(tc.tile_pool(name="obuf", bufs=2))
        psA = ctx.enter_context(tc.tile_pool(name="psA", bufs=1, space="PSUM"))
        psB = ctx.enter_context(tc.tile_pool(name="psB", bufs=1, space="PSUM"))
        ps_s2 = ctx.enter_context(tc.tile_pool(name="ps_s2", bufs=2, space="PSUM"))
        ps_z = ctx.enter_context(tc.tile_pool(name="ps_z", bufs=1, space="PSUM"))
        ps_o = ctx.enter_context(tc.tile_pool(name="ps_o", bufs=1, space="PSUM"))

        masks = consts.tile([P, NG * GW], f16)
        nc.sync.dma_start(masks[:], mk)
        ident = consts.tile([P, DH], f16)
        nc.sync.dma_start(ident[0:DH, :], idin)
        nc.sync.dma_start(ident[DH:P, :], idin)
        if use_bias:
            ones = consts.tile([1, GW], f16)
            nc.sync.dma_start(ones[:], onesr)

        # outproj work queue: [zaug, recip-slot (filled late), p, g, wo_sb]
        pending = []

        def flush_outproj(drain=False):
            # transposed outproj: W_O blocks stationary (reused, cheap
            # LDWEIGHTS), zaug moving; output is o^T = W_O^T z_un per m-block,
            # evicted as a plain cast -- normalization happens on the host
            zaug_, sums_, p_, g_, wo_sb_ = pending.pop(0)
            ob = obuf.tile([P, NCH * GW], f16, tag="ob")
            for mb in range(NCH):
                if drain and mb % 2 == 1:
                    # at drain time the scores banks are free; alternate
                    # into them so the next matmul never waits an evict
                    o_ps = ps_s2.tile([P, 2 * GW], f32, tag="s2",
                                      name="o_ps")
                else:
                    o_ps = ps_o.tile([P, GW], f32, tag="o", name="o_ps")
                nc.tensor.matmul(
                    o_ps[:, 0:GW],
                    lhsT=wo_sb_[:, mb * P:(mb + 1) * P],
                    rhs=zaug_[:],
                    start=True,
                    stop=True,
                )
                dst = ob[:, mb * GW:(mb + 1) * GW]
                if mb % 3 == 2:
                    nc.scalar.copy(dst, o_ps[:, 0:GW])
                else:
                    nc.vector.tensor_copy(dst, o_ps[:, 0:GW])
                if drain and mb == 2:
                    # overlap the first half of the final transfer with the
                    # remaining evictions
                    nc.scalar.dma_start(
                        outT[p_, g_, :, 0:3 * GW], ob[:, 0:3 * GW])
                yield
                yield
            if drain:
                # Scalar queue: program order puts this right after the last
                # eviction; the congested Sync queue would delay it
                nc.scalar.dma_start(
                    outT[p_, g_, :, 3 * GW:], ob[:, 3 * GW:])
            else:
                nc.sync.dma_start(outT[p_, g_], ob[:])
            nc.gpsimd.dma_start(dnm[p_, g_], sums_[:])

        def gen_proj(p, out):
            """DMAs + projections for pair p. Yields "dma" once after DMA
            emission (prime point), "head" after K/Q quarter 0."""
            # head-critical DMAs first: wk + xk quarter 0, wq + xq quarter 0.
            # For pair 0 the q-side goes on the Sync queue so both quarters'
            # transfers issue in parallel (startup latency).
            qeng = nc.sync if p == 0 else nc.gpsimd
            wk_sb = wpool.tile([P, NCH * DH], f16, tag="wk")
            nc.gpsimd.dma_start(wk_sb[:], wk[p])
            xk_sb = xin8.tile([P, NQ * NCH * GW], f8, tag="xk")
            xk_v = xk_sb[:].rearrange("p (q c s) -> p q c s", q=NQ, c=NCH)
            nc.gpsimd.dma_start(xk_v[:, 0], xk[p, :, 0])
            wq_sb = wpool.tile([P, NCH * DH], f16, tag="wq")
            qeng.dma_start(wq_sb[:], wq[p])
            xq_sb = xin8.tile([P, NQ * NCH * GW], f8, tag="xq")
            xq_v = xq_sb[:].rearrange("p (q c s) -> p q c s", q=NQ, c=NCH)
            qeng.dma_start(xq_v[:, 0], xq[p, :, 0])
            wv_sb = wpool.tile([P, NCH * DH], f16, tag="wv")
            nc.gpsimd.dma_start(wv_sb[:], wv[p])
            wo_sb = wpool.tile([VW, DM], f16, tag="wo")
            nc.gpsimd.dma_start(wo_sb[:], wo[p])
            if use_bias:
                bq_sb = wpool.tile([1, DH], f16, tag="bq")
                nc.gpsimd.dma_start(bq_sb[:], bq[p])
                bk_sb = wpool.tile([1, DH], f16, tag="bk")
                nc.gpsimd.dma_start(bk_sb[:], bk[p])
                bv_sb = wpool.tile([1, DH], f16, tag="bv")
                nc.gpsimd.dma_start(bv_sb[:], bv[p])
            out["wo"] = wo_sb

            xv_sb = xin16.tile([P, NQ * NCH * GW], f16, tag="xv")
            xv_v = xv_sb[:].rearrange("p (q c s) -> p q c s", q=NQ, c=NCH)
            for q in range(1, NQ):
                nc.gpsimd.dma_start(xk_v[:, q], xk[p, :, q])
                nc.gpsimd.dma_start(xq_v[:, q], xq[p, :, q])
            for q in range(NQ):
                nc.gpsimd.dma_start(xv_v[:, q], xv[p, :, q])
            vaug = prj.tile([P, NSK * VW], f16, tag="vaug")
            nc.sync.dma_start(
                vaug[:].rearrange("p (i w) -> p i w", w=VW)[:, :, DH:VW], onesc
            )
            out["vaug"] = vaug
            out["vt_done"] = 0
            out["qk_done"] = 0
            qt = prj.tile([P, S], f16, tag="qt")
            kt = prj.tile([P, S], f16, tag="kt")
            yield "dma"

            def qk_quarter(q):
                # col-packed: Q in PE columns 0:63 (psA), K in 64:127 (psB) --
                # two open accumulation groups must use separate banks
                pa = psA.tile([P, GW], f32, tag="u", name="qk_psa")
                pb = psB.tile([P, GW], f32, tag="u", name="qk_psb")
                for c in range(NCH):
                    nc.tensor.matmul(
                        pa[0:DH, :],
                        lhsT=wq_sb[:, c * DH:(c + 1) * DH],
                        rhs=xq_v[:, q, c, :],
                        start=(c == 0),
                        stop=(c == NCH - 1) and not use_bias,
                        tile_position=(0, 0),
                    )
                    nc.tensor.matmul(
                        pb[DH:P, :],
                        lhsT=wk_sb[:, c * DH:(c + 1) * DH],
                        rhs=xk_v[:, q, c, :],
                        start=(c == 0),
                        stop=(c == NCH - 1) and not use_bias,
                        tile_position=(0, 64),
                    )
                    yield
                if use_bias:
                    nc.tensor.matmul(
                        pa[0:DH, :], lhsT=bq_sb[:], rhs=ones[:],
                        start=False, stop=True, tile_position=(0, 0))
                    nc.tensor.matmul(
                        pb[DH:P, :], lhsT=bk_sb[:], rhs=ones[:],
                        start=False, stop=True, tile_position=(0, 64))
                    yield
                qs = slice(q * GW, (q + 1) * GW)
                nc.vector.tensor_copy(qt[0:DH, qs], pa[0:DH, :])
                # duplicate to partitions 64:128 off-engine (SBUF->SBUF DMA)
                nc.gpsimd.dma_start(qt[DH:P, qs], qt[0:DH, qs])
                src = pb[DH:P, :].rearrange(
                    "p (b two c) -> p b two c", b=2, two=2)
                de = kt[0:DH, qs].rearrange(
                    "p (b two c) -> p b two c", b=2, two=2)
                do = kt[DH:P, qs].rearrange(
                    "p (b two c) -> p b two c", b=2, two=2)
                nc.scalar.copy(de[:, :, 0, :], src[:, :, 0, :])
                nc.vector.tensor_copy(do[:, :, 1, :], src[:, :, 1, :])
                yield
                yield

            yield from qk_quarter(0)
            out["qt"] = qt
            out["kt"] = kt
            out["qk_done"] = 1
            yield "head"
            for q in range(1, NQ):
                yield from qk_quarter(q)
                out["qk_done"] = q + 1

            # ---- V projection (col-packed S-quarter pairs) + transposes ----
            vt = prj.tile([DH, S], f16, tag="vt")
            for qp in range(2):
                qa, qb = 2 * qp, 2 * qp + 1
                pa = psA.tile([P, GW], f32, tag="u", name="v_psa")
                pb = psB.tile([P, GW], f32, tag="u", name="v_psb")
                for c in range(NCH):
                    nc.tensor.matmul(
                        pa[0:DH, :],
                        lhsT=wv_sb[:, c * DH:(c + 1) * DH],
                        rhs=xv_v[:, qa, c, :],
                        start=(c == 0),
                        stop=(c == NCH - 1) and not use_bias,
                        tile_position=(0, 0),
                    )
                    nc.tensor.matmul(
                        pb[DH:P, :],
                        lhsT=wv_sb[:, c * DH:(c + 1) * DH],
                        rhs=xv_v[:, qb, c, :],
                        start=(c == 0),
                        stop=(c == NCH - 1) and not use_bias,
                        tile_position=(0, 64),
                    )
                    yield
                if use_bias:
                    nc.tensor.matmul(
                        pa[0:DH, :], lhsT=bv_sb[:], rhs=ones[:],
                        start=False, stop=True, tile_position=(0, 0))
                    nc.tensor.matmul(
                        pb[DH:P, :], lhsT=bv_sb[:], rhs=ones[:],
                        start=False, stop=True, tile_position=(0, 64))
                    yield
                nc.vector.tensor_copy(vt[:, qa * GW:(qa + 1) * GW], pa[0:DH, :])
                nc.scalar.copy(vt[:, qb * GW:(qb + 1) * GW], pb[DH:P, :])
                yield
                # transpose the four ready sk blocks of each finished quarter
                for i in range(8 * qp, 8 * qp + 8):
                    pool = psA if i % 2 == 0 else psB
                    tp = pool.tile([P, DH], f16, tag="u", name="vtr_ps")
                    nc.tensor.transpose(
                        tp[:], vt[:, i * P:(i + 1) * P], ident[0:DH, :]
                    )
                    nc.vector.tensor_copy(vaug[:, i * VW:i * VW + DH], tp[:])
                    out["vt_done"] = i + 1
                    yield

        def gen_att(p, tiles):
            # the background stream emits this pair's projections; spin until
            # the QK tiles exist (each yield advances the background by one)
            while "qt" not in tiles:
                yield
            qt, kt, wo_sb = tiles["qt"], tiles["kt"], tiles["wo"]
            vaug = tiles["vaug"]

            for g in range(NG):
                # emission-order guard: scores of group g read qt quarter g
                # and kt quarters 0..g; their evictions must be emitted first
                while tiles["qk_done"] <= g:
                    yield
                gs = slice(g * GW, (g + 1) * GW)
                nsk = 4 * (g + 1)
                zctx = {"ps": None}

                def emit_scores_pair(ip, g=g, gs=gs):
                    s_ps = ps_s2.tile([P, 2 * GW], f32, tag="s2")
                    nc.tensor.matmul(
                        s_ps[:, 0:GW],
                        lhsT=kt[0:DH, ip * P:(ip + 1) * P],
                        rhs=qt[0:DH, gs],
                        start=True, stop=True,
                        tile_position=(0, 0),
                    )
                    nc.tensor.matmul(
                        s_ps[:, GW:2 * GW],
                        lhsT=kt[DH:P, (ip + 1) * P:(ip + 2) * P],
                        rhs=qt[DH:P, gs],
                        start=True, stop=True,
                        tile_position=(64, 0),
                    )
                    e_sb = expp.tile([P, 2 * GW], f16, tag="exp")
                    nc.scalar.activation(e_sb[:], s_ps[:], AF.Exp, scale=0.125)
                    if ip >= 4 * g:
                        j = ip - 4 * g
                        nc.vector.tensor_mul(
                            e_sb[:], e_sb[:], masks[:, j * GW:(j + 2) * GW])
                    return e_sb

                def emit_z(ip, e_use, nsk=nsk, zctx=zctx):
                    for k in range(2):
                        i = ip + k
                        # emission-order guard: the transpose writing vaug
                        # block i must be EMITTED before this read (the Tile
                        # dep tracker only sees already-emitted writers)
                        while tiles["vt_done"] <= i:
                            yield
                        if zctx["ps"] is None:
                            zctx["ps"] = ps_z.tile(
                                [VW, GW], f32, tag="z", name="z_ps")
                        nc.tensor.matmul(
                            zctx["ps"][:],
                            lhsT=vaug[:, i * VW:(i + 1) * VW],
                            rhs=e_use[:, k * GW:(k + 1) * GW],
                            start=(i == 0),
                            stop=(i == nsk - 1),
                        )
                        yield

                # z runs lag-2 behind scores
                eq = []
                for ip in range(0, nsk, 2):
                    eq.append((ip, emit_scores_pair(ip)))
                    yield
                    # eager flush: the previous group's outproj goes out
                    # right away (keeps the tail short)
                    if pending:
                        yield from flush_outproj()
                    if len(eq) > 2:
                        ip0, e0 = eq.pop(0)
                        yield from emit_z(ip0, e0)
                while eq:
                    ip0, e0 = eq.pop(0)
                    yield from emit_z(ip0, e0)

                z_ps = zctx["ps"]
                zaug = smal.tile([VW, GW], f16, tag="zaug")
                nc.scalar.copy(zaug[:], z_ps[:])
                sums0 = smal.tile([1, GW], f16, tag="sums0")
                nc.vector.tensor_copy(sums0[:], z_ps[DH:VW, :])
                pending.append([zaug, sums0, p, g, wo_sb])

        def interleave(a, b, bpulls=2):
            """Pull a once and b `bpulls` times per cycle until a exhausts;
            b is a shared background stream that survives across calls.
            Front-loading b keeps the PE instruction stream dense (HAM)."""
            a_live = True
            while a_live:
                try:
                    next(a)
                except StopIteration:
                    a_live = False
                for _ in range(bpulls):
                    if b is None:
                        break
                    try:
                        next(b)
                    except StopIteration:
                        b = None
            return b

        def chain(*gens):
            for g in gens:
                yield from g

        tiles = [{} for _ in range(PPC)]
        gens = [gen_proj(p, tiles[p]) for p in range(PPC)]
        # prime pair-0 DMAs, then warm the HAM clock gate with throwaway
        # matmuls on the masks tile (lands ~2us in) so the real projections
        # start at 2.4 GHz instead of ramping from 1.2 until ~36us
        next(gens[0])
        # warmup reads an uninitialized scratch tile: no DMA dependency, so
        # the matmuls issue as soon as the engines come up (~2us) and the HAM
        # clock gate is released before the first real projection
        scratch = consts.tile([P, GW], f16)
        nc.vector.memset(scratch[:], 0)
        warm_ps = psA.tile([DH, GW], f32, tag="u", name="warm_ps")
        for r in range(10):
            nc.tensor.matmul(
                warm_ps[:],
                lhsT=scratch[:, 0:DH],
                rhs=scratch[:],
                start=(r == 0),
                stop=(r == 9),
            )
        # finish the pair-0 head (first K/Q quarter) serially
        for v in gens[0]:
            if v == "head":
                break
        # background: rest of proj(0), then proj(1), proj(2)
        bg = chain(*gens)
        for p in range(PPC):
            # front-load projections during pair 0 (dense PE warms the HAM
            # clock gate); 1:1 after so emitted proj matmuls never get far
            # enough ahead of their input DMAs to head-block the PE queue
            bg = interleave(gen_att(p, tiles[p]), bg, bpulls=1)
        while bg is not None:
            try:
                next(bg)
            except StopIteration:
                bg = None
        while pending:
            for _ in flush_outproj(drain=True):
                pass

    nc.compile()
    return nc


def get_nc(use_bias=False):
    if use_bias not in _NC_CACHE:
        _NC_CACHE[use_bias] = _build_bass(use_bias)
    return _NC_CACHE[use_bias]


def _pairs_for_core(c):
    return [(idx // H, idx % H) for idx in range(c * PPC, (c + 1) * PPC)]


def make_masks():
    # mask[p, (j c)] = 1.0 iff key pos 128*j + p <= query pos c (within block)
    j = np.arange(NG)[None, :, None]
    p = np.arange(P)[:, None, None]
    f = np.arange(GW)[None, None, :]
    return (f >= P * j + p).astype(NP_W).reshape(P, NG * GW)


def _xT_quarters(x, b, h, np_dt):
    # [S, DM] -> [DM, S] -> [P, NQ, NCH, GW] (partition-major quarter blocks)
    xt = x[b, :, h, :].T.astype(np_dt)          # [DM, S]
    xt = xt.reshape(NCH, P, NQ, GW)
    return np.ascontiguousarray(xt.transpose(1, 2, 0, 3))


def make_in_maps(inputs, use_bias):
    xq = np.asarray(inputs["normalized_resid_pre_q"], dtype=np.float32)
    xk = np.asarray(inputs["normalized_resid_pre_k"], dtype=np.float32)
    xv = np.asarray(inputs["normalized_resid_pre_v"], dtype=np.float32)
    W_Q = np.asarray(inputs["W_Q"], dtype=np.float32)
    W_K = np.asarray(inputs["W_K"], dtype=np.float32)
    W_V = np.asarray(inputs["W_V"], dtype=np.float32)
    b_Q = np.asarray(inputs["b_Q"], dtype=np.float32)
    b_K = np.asarray(inputs["b_K"], dtype=np.float32)
    b_V = np.asarray(inputs["b_V"], dtype=np.float32)
    W_O = np.asarray(inputs["W_O"], dtype=np.float32)
    b_O = np.asarray(inputs["b_O"], dtype=np.float32)

    def w_pmajor(W):
        # [DM, DH] -> [NCH, P, DH] -> [P, NCH*DH]
        w = W.astype(NP_W).reshape(NCH, P, DH)
        return np.ascontiguousarray(w.transpose(1, 0, 2)).reshape(P, NCH * DH)

    masks = make_masks()
    onesc = np.ones((P, NSK, 1), NP_W)
    ident64 = np.eye(DH, dtype=NP_W)
    in_maps = []
    for c in range(NCORES):
        pairs = _pairs_for_core(c)
        m = {
            "xqT": np.stack([_xT_quarters(xq, b, h, NP_X8) for b, h in pairs]),
            "xkT": np.stack([_xT_quarters(xk, b, h, NP_X8) for b, h in pairs]),
            "xvT": np.stack([_xT_quarters(xv, b, h, NP_W) for b, h in pairs]),
            "wq": np.stack([w_pmajor(W_Q[h]) for b, h in pairs]),
            "wk": np.stack([w_pmajor(W_K[h]) for b, h in pairs]),
            "wv": np.stack([w_pmajor(W_V[h]) for b, h in pairs]),
            "wo": np.stack(
                [np.concatenate([W_O[h], (b_O / H)[None, :]], axis=0).astype(NP_W)
                 for b, h in pairs]),
            "masks": masks,
            "ones_col": onesc,
            "ident64": ident64,
        }
        if use_bias:
            m["bq"] = np.stack([b_Q[h][None, :].astype(NP_W) for b, h in pairs])
            m["bk"] = np.stack([b_K[h][None, :].astype(NP_W) for b, h in pairs])
            m["bv"] = np.stack([b_V[h][None, :].astype(NP_W) for b, h in pairs])
            m["ones_row"] = np.ones((1, GW), NP_W)
        in_maps.append(m)
    return in_maps


def needs_bias(inputs):
    return any(
        np.any(np.asarray(inputs[k])) for k in ("b_Q", "b_K", "b_V")
    )


def assemble_output(results):
    out = np.empty((B, S, H, DM), np.float32)
    for c in range(NCORES):
        for j, (b, h) in enumerate(_pairs_for_core(c)):
            # outT[j]: [NG, P(m within block), NCH, GW] transposed
            # unnormalized; denoms[j]: [NG, 1, GW]
            o = results[c]["outT"][j].astype(np.float32)
            o = o.reshape(NG, P, NCH, GW).transpose(0, 3, 2, 1).reshape(S, DM)
            d = results[c]["denoms"][j].astype(np.float32).reshape(S, 1)
            out[b, :, h, :] = o / d
    return out


def kernel(**inputs):
    from concourse import bass_utils

    use_bias = needs_bias(inputs)
    nc = get_nc(use_bias)
    in_maps = make_in_maps(inputs, use_bias)
    res = bass_utils.run_bass_kernel_spmd(nc, in_maps, core_ids=list(range(NCORES)))
    return assemble_output(res.results)


# revision 52
# speedup vs baseline: 1.0347x; 1.0042x over previous
"""Bass/Trainium2 kernel for per-head attention (B=2, S=2048, H=12, DM=768, DH=64).

Sharding: 24 (batch, head) pairs -> 8 cores x 3 pairs. Host pre-transposes the
per-pair activations to [DM, S] in partition-major quarter-blocked layout
[P, NQ, NCH, GW] (one contiguous 3KB DMA line per partition per quarter);
xq/xk are fp8e3 (e3m4), xv fp16 (V-path quantization propagates ~1:1 to the
output; the QK path is dampened by softmax). Weights are fp16; matmuls mix
fp16 lhsT with fp8 rhs at full rate.

Per pair:
  Q^T/K^T/V^T computed per S-quarter as serial M=64 matmuls accumulating 6
  d_model chunks in a single PSUM bank (col-packed concurrent tiles only
  co-stream ~25% of the time -- LDWEIGHTS with a shared row group cannot be
  pulled ahead -- and the second bank is better spent on the outproj).
  K^T is evicted split by sk-block parity (even blocks -> partitions 0:64,
  odd -> 64:128) so score row-packing needs no K duplication; Q^T is evicted
  once and duplicated to partitions 64:128 by an SBUF->SBUF DMA (off the
  compute engines). scores^T pair = two K=64 matmuls row-packed at rows 0/64.
  P_u = exp(0.125 scores^T) on ACT; diagonal blocks masked in place on DVE.
  Z runs two score-pairs behind (lag-2) so the in-order PE queue never parks
  on an exp chain; Zaug (ones column -> denominators in row 64) accumulates in
  one bank. The output projection is TRANSPOSED: W_O m-blocks are the
  stationary operand (64-col LDWEIGHTS instead of reloading a 128-col zaug
  block per matmul) and zaug the moving one, producing unnormalized
  o^T = W_O^T z_un evicted as plain casts. The denominator row ships to the
  host, which divides (exact same math; the ones-row times b_O/H keeps the
  bias exact).

Scheduling: pair 0 emits DMAs + HAM-warmup matmuls + K/Q quarter-0 serially,
then attention(p) interleaves 1:1 with a background stream = [rest of pair
p's projections, pair p+1's projections], so attention starts as soon as the
first quarter lands. Z emission is gated on a vt_done counter and scores on a
qk_done counter (the Tile dep tracker only sees already-emitted writers).
Input DMAs issue from the otherwise-idle GPSIMD queue so their ring-slot
waits never block the output DMAs (Sync; Scalar for the drain tail).
"""

import numpy as np
import ml_dtypes

B, S, H, DM, DH = 2, 2048, 12, 768, 64
P = 128
NCORES = 8
PPC = (B * H) // NCORES   # pairs per core = 3
NCH = DM // P             # 6 d_model chunks
NG = 4                    # sq groups
GW = S // NG              # 512
NSK = S // P              # 16 sk tiles
VW = DH + 1               # 65 (V augmented with ones column)
NQ = 4                    # S quarters (= NG)
MH = DM // 2              # outproj m-half = 384
NT = GW // P              # q tiles per group = 4

NP_W = np.float16
NP_X8 = ml_dtypes.float8_e3m4

_NC_CACHE = {}


def _build_bass(use_bias):
    import concourse.mybir as mybir
    import concourse.tile as tile
    from concourse import bacc
    from contextlib import ExitStack

    dt = mybir.dt
    f32 = dt.float32
    f16 = dt.float16
    f8 = dt.float8e3
    AF = mybir.ActivationFunctionType

    nc = bacc.Bacc("TRN2", target_bir_lowering=False, debug=False)

    # x layouts: [pair][partition][quarter][chunk][col] (3KB DMA lines)
    xq = nc.dram_tensor("xqT", [PPC, P, NQ, NCH, GW], f8, kind="ExternalInput").ap()
    xk = nc.dram_tensor("xkT", [PPC, P, NQ, NCH, GW], f8, kind="ExternalInput").ap()
    xv = nc.dram_tensor("xvT", [PPC, P, NQ, NCH, GW], f16, kind="ExternalInput").ap()
    # weights: [pair][partition][chunk][e] (p-major, single DMA line/partition)
    wq = nc.dram_tensor("wq", [PPC, P, NCH * DH], f16, kind="ExternalInput").ap()
    wk = nc.dram_tensor("wk", [PPC, P, NCH * DH], f16, kind="ExternalInput").ap()
    wv = nc.dram_tensor("wv", [PPC, P, NCH * DH], f16, kind="ExternalInput").ap()
    if use_bias:
        bq = nc.dram_tensor("bq", [PPC, 1, DH], f16, kind="ExternalInput").ap()
        bk = nc.dram_tensor("bk", [PPC, 1, DH], f16, kind="ExternalInput").ap()
        bv = nc.dram_tensor("bv", [PPC, 1, DH], f16, kind="ExternalInput").ap()
        onesr = nc.dram_tensor(
            "ones_row", [1, GW], f16, kind="ExternalInput").ap()
    wo = nc.dram_tensor("wo", [PPC, VW, DM], f16, kind="ExternalInput").ap()
    mk = nc.dram_tensor("masks", [P, NG * GW], f16, kind="ExternalInput").ap()
    onesc = nc.dram_tensor("ones_col", [P, NSK, 1], f16, kind="ExternalInput").ap()
    idin = nc.dram_tensor("ident64", [DH, DH], f16, kind="ExternalInput").ap()
    # out (TRANSPOSED, unnormalized): [pair][group][partition(m within
    # block)][mb*GW + q]; host divides by the denominators
    outT = nc.dram_tensor("outT", [PPC, NG, P, NCH * GW], f16,
                          kind="ExternalOutput").ap()
    dnm = nc.dram_tensor("denoms", [PPC, NG, 1, GW], f16,
                         kind="ExternalOutput").ap()

    with tile.TileContext(nc) as tc, ExitStack() as ctx:
        consts = ctx.enter_context(tc.tile_pool(name="consts", bufs=1))
        wpool = ctx.enter_context(tc.tile_pool(name="wpool", bufs=2))
        xin8 = ctx.enter_context(tc.tile_pool(name="xin8", bufs=2))
        xin16 = ctx.enter_context(tc.tile_pool(name="xin16", bufs=2))
        prj = ctx.enter_context(tc.tile_pool(name="prj", bufs=2))
        expp = ctx.enter_context(tc.tile_pool(name="expp", bufs=8))
        smal = ctx.enter_context(tc.tile_pool(name="smal", bufs=4))
        obuf = ctx.enter_context(tc.tile_pool(name="obuf", bufs=2))
        psA = ctx.enter_context(tc.tile_pool(name="psA", bufs=1, space="PSUM"))
        psB = ctx.enter_context(tc.tile_pool(name="psB", bufs=1, space="PSUM"))
        ps_s2 = ctx.enter_context(tc.tile_pool(name="ps_s2", bufs=2, space="PSUM"))
        ps_z = ctx.enter_context(tc.tile_pool(name="ps_z", bufs=1, space="PSUM"))
        ps_o = ctx.enter_context(tc.tile_pool(name="ps_o", bufs=1, space="PSUM"))

        masks = consts.tile([P, NG * GW], f16)
        nc.sync.dma_start(masks[:], mk)
        ident = consts.tile([P, DH], f16)
        nc.sync.dma_start(ident[0:DH, :], idin)
        nc.sync.dma_start(ident[DH:P, :], idin)
        if use_bias:
            ones = consts.tile([1, GW], f16)
            nc.sync.dma_start(ones[:], onesr)

        # outproj work queue: [zaug, recip-slot (filled late), p, g, wo_sb]
        pending = []

        def flush_outproj(drain=False):
            # transposed outproj: W_O blocks stationary (reused, cheap
            # LDWEIGHTS), zaug moving; output is o^T = W_O^T z_un per m-block,
            # evicted as a plain cast -- normalization happens on the host
            zaug_, sums_, p_, g_, wo_sb_ = pending.pop(0)
            ob = obuf.tile([P, NCH * GW], f16, tag="ob")
            for mb in range(NCH):
                if drain and mb % 2 == 1:
                    # at drain time the scores banks are free; alternate
                    # into them so the next matmul never waits an evict
                    o_ps = ps_s2.tile([P, 2 * GW], f32, tag="s2",
                                      name="o_ps")
                else:
                    o_ps = ps_o.tile([P, GW], f32, tag="o", name="o_ps")
                nc.tensor.matmul(
                    o_ps[:, 0:GW],
                    lhsT=wo_sb_[:, mb * P:(mb + 1) * P],
                    rhs=zaug_[:],
                    start=True,
                    stop=True,
                )
                dst = ob[:, mb * GW:(mb + 1) * GW]
                if mb % 3 == 2:
                    nc.scalar.copy(dst, o_ps[:, 0:GW])
                else:
                    nc.vector.tensor_copy(dst, o_ps[:, 0:GW])
                if drain and mb == 2:
                    # overlap the first half of the final transfer with the
                    # remaining evictions
                    nc.gpsimd.dma_start(
                        outT[p_, g_, :, 0:3 * GW], ob[:, 0:3 * GW])
                yield
                yield
            # GPSIMD queue: its input-DMA waits are short and it is idle in
            # the late phase, so output transfers are never queued behind the
            # Sync semaphore backlog (which stalls the ob ring) or the Scalar
            # compute backlog (which delayed the drain by ~9us)
            if drain:
                nc.gpsimd.dma_start(
                    outT[p_, g_, :, 3 * GW:], ob[:, 3 * GW:])
            else:
                nc.gpsimd.dma_start(outT[p_, g_], ob[:])
            nc.gpsimd.dma_start(dnm[p_, g_], sums_[:])

        def gen_proj(p, out):
            """DMAs + projections for pair p. Yields "dma" once after DMA
            emission (prime point), "head" after K/Q quarter 0."""
            # head-critical DMAs first: wk + xk quarter 0, wq + xq quarter 0.
            # For pair 0 the q-side goes on the Sync queue so both quarters'
            # transfers issue in parallel (startup latency).
            qeng = nc.sync if p == 0 else nc.gpsimd
            wk_sb = wpool.tile([P, NCH * DH], f16, tag="wk")
            nc.gpsimd.dma_start(wk_sb[:], wk[p])
            xk_sb = xin8.tile([P, NQ * NCH * GW], f8, tag="xk")
            xk_v = xk_sb[:].rearrange("p (q c s) -> p q c s", q=NQ, c=NCH)
            nc.gpsimd.dma_start(xk_v[:, 0], xk[p, :, 0])
            wq_sb = wpool.tile([P, NCH * DH], f16, tag="wq")
            qeng.dma_start(wq_sb[:], wq[p])
            xq_sb = xin8.tile([P, NQ * NCH * GW], f8, tag="xq")
            xq_v = xq_sb[:].rearrange("p (q c s) -> p q c s", q=NQ, c=NCH)
            qeng.dma_start(xq_v[:, 0], xq[p, :, 0])
            wv_sb = wpool.tile([P, NCH * DH], f16, tag="wv")
            nc.gpsimd.dma_start(wv_sb[:], wv[p])
            wo_sb = wpool.tile([VW, DM], f16, tag="wo")
            nc.gpsimd.dma_start(wo_sb[:], wo[p])
            if use_bias:
                bq_sb = wpool.tile([1, DH], f16, tag="bq")
                nc.gpsimd.dma_start(bq_sb[:], bq[p])
                bk_sb = wpool.tile([1, DH], f16, tag="bk")
                nc.gpsimd.dma_start(bk_sb[:], bk[p])
                bv_sb = wpool.tile([1, DH], f16, tag="bv")
                nc.gpsimd.dma_start(bv_sb[:], bv[p])
            out["wo"] = wo_sb

            xv_sb = xin16.tile([P, NQ * NCH * GW], f16, tag="xv")
            xv_v = xv_sb[:].rearrange("p (q c s) -> p q c s", q=NQ, c=NCH)
            for q in range(1, NQ):
                nc.gpsimd.dma_start(xk_v[:, q], xk[p, :, q])
                nc.gpsimd.dma_start(xq_v[:, q], xq[p, :, q])
            for q in range(NQ):
                nc.gpsimd.dma_start(xv_v[:, q], xv[p, :, q])
            vaug = prj.tile([P, NSK * VW], f16, tag="vaug")
            nc.sync.dma_start(
                vaug[:].rearrange("p (i w) -> p i w", w=VW)[:, :, DH:VW], onesc
            )
            out["vaug"] = vaug
            out["vt_done"] = 0
            out["qk_done"] = 0
            qt = prj.tile([P, S], f16, tag="qt")
            kt = prj.tile([P, S], f16, tag="kt")
            yield "dma"

            def qk_quarter(q):
                # col-packed: Q in PE columns 0:63 (psA), K in 64:127 (psB) --
                # two open accumulation groups must use separate banks
                pa = psA.tile([P, GW], f32, tag="u", name="qk_psa")
                pb = psB.tile([P, GW], f32, tag="u", name="qk_psb")
                for c in range(NCH):
                    nc.tensor.matmul(
                        pa[0:DH, :],
                        lhsT=wq_sb[:, c * DH:(c + 1) * DH],
                        rhs=xq_v[:, q, c, :],
                        start=(c == 0),
                        stop=(c == NCH - 1) and not use_bias,
                        tile_position=(0, 0),
                    )
                    nc.tensor.matmul(
                        pb[DH:P, :],
                        lhsT=wk_sb[:, c * DH:(c + 1) * DH],
                        rhs=xk_v[:, q, c, :],
                        start=(c == 0),
                        stop=(c == NCH - 1) and not use_bias,
                        tile_position=(0, 64),
                    )
                    yield
                if use_bias:
                    nc.tensor.matmul(
                        pa[0:DH, :], lhsT=bq_sb[:], rhs=ones[:],
                        start=False, stop=True, tile_position=(0, 0))
                    nc.tensor.matmul(
                        pb[DH:P, :], lhsT=bk_sb[:], rhs=ones[:],
                        start=False, stop=True, tile_position=(0, 64))
                    yield
                qs = slice(q * GW, (q + 1) * GW)
                nc.vector.tensor_copy(qt[0:DH, qs], pa[0:DH, :])
                # duplicate to partitions 64:128 off-engine (SBUF->SBUF DMA)
                nc.gpsimd.dma_start(qt[DH:P, qs], qt[0:DH, qs])
                src = pb[DH:P, :].rearrange(
                    "p (b two c) -> p b two c", b=2, two=2)
                de = kt[0:DH, qs].rearrange(
                    "p (b two c) -> p b two c", b=2, two=2)
                do = kt[DH:P, qs].rearrange(
                    "p (b two c) -> p b two c", b=2, two=2)
                nc.scalar.copy(de[:, :, 0, :], src[:, :, 0, :])
                nc.vector.tensor_copy(do[:, :, 1, :], src[:, :, 1, :])
                yield
                yield

            yield from qk_quarter(0)
            out["qt"] = qt
            out["kt"] = kt
            out["qk_done"] = 1
            yield "head"
            for q in range(1, NQ):
                yield from qk_quarter(q)
                out["qk_done"] = q + 1

            # ---- V projection (col-packed S-quarter pairs) + transposes ----
            vt = prj.tile([DH, S], f16, tag="vt")
            for qp in range(2):
                qa, qb = 2 * qp, 2 * qp + 1
                pa = psA.tile([P, GW], f32, tag="u", name="v_psa")
                pb = psB.tile([P, GW], f32, tag="u", name="v_psb")
                for c in range(NCH):
                    nc.tensor.matmul(
                        pa[0:DH, :],
                        lhsT=wv_sb[:, c * DH:(c + 1) * DH],
                        rhs=xv_v[:, qa, c, :],
                        start=(c == 0),
                        stop=(c == NCH - 1) and not use_bias,
                        tile_position=(0, 0),
                    )
                    nc.tensor.matmul(
                        pb[DH:P, :],
                        lhsT=wv_sb[:, c * DH:(c + 1) * DH],
                        rhs=xv_v[:, qb, c, :],
                        start=(c == 0),
                        stop=(c == NCH - 1) and not use_bias,
                        tile_position=(0, 64),
                    )
                    yield
                if use_bias:
                    nc.tensor.matmul(
                        pa[0:DH, :], lhsT=bv_sb[:], rhs=ones[:],
                        start=False, stop=True, tile_position=(0, 0))
                    nc.tensor.matmul(
                        pb[DH:P, :], lhsT=bv_sb[:], rhs=ones[:],
                        start=False, stop=True, tile_position=(0, 64))
                    yield
                nc.vector.tensor_copy(vt[:, qa * GW:(qa + 1) * GW], pa[0:DH, :])
                nc.scalar.copy(vt[:, qb * GW:(qb + 1) * GW], pb[DH:P, :])
                yield
                # transpose the four ready sk blocks of each finished quarter
                for i in range(8 * qp, 8 * qp + 8):
                    pool = psA if i % 2 == 0 else psB
                    tp = pool.tile([P, DH], f16, tag="u", name="vtr_ps")
                    nc.tensor.transpose(
                        tp[:], vt[:, i * P:(i + 1) * P], ident[0:DH, :]
                    )
                    nc.vector.tensor_copy(vaug[:, i * VW:i * VW + DH], tp[:])
                    out["vt_done"] = i + 1
                    yield

        def gen_att(p, tiles):
            # the background stream emits this pair's projections; spin until
            # the QK tiles exist (each yield advances the background by one)
            while "qt" not in tiles:
                yield
            qt, kt, wo_sb = tiles["qt"], tiles["kt"], tiles["wo"]
            vaug = tiles["vaug"]

            for g in range(NG):
                # emission-order guard: scores of group g read qt quarter g
                # and kt quarters 0..g; their evictions must be emitted first
                while tiles["qk_done"] <= g:
                    yield
                gs = slice(g * GW, (g + 1) * GW)
                nsk = 4 * (g + 1)
                zctx = {"ps": None}

                def emit_scores_pair(ip, g=g, gs=gs):
                    s_ps = ps_s2.tile([P, 2 * GW], f32, tag="s2")
                    nc.tensor.matmul(
                        s_ps[:, 0:GW],
                        lhsT=kt[0:DH, ip * P:(ip + 1) * P],
                        rhs=qt[0:DH, gs],
                        start=True, stop=True,
                        tile_position=(0, 0),
                    )
                    nc.tensor.matmul(
                        s_ps[:, GW:2 * GW],
                        lhsT=kt[DH:P, (ip + 1) * P:(ip + 2) * P],
                        rhs=qt[DH:P, gs],
                        start=True, stop=True,
                        tile_position=(64, 0),
                    )
                    e_sb = expp.tile([P, 2 * GW], f16, tag="exp")
                    nc.scalar.activation(e_sb[:], s_ps[:], AF.Exp, scale=0.125)
                    if ip >= 4 * g:
                        j = ip - 4 * g
                        nc.vector.tensor_mul(
                            e_sb[:], e_sb[:], masks[:, j * GW:(j + 2) * GW])
                    return e_sb

                def emit_z(ip, e_use, nsk=nsk, zctx=zctx):
                    for k in range(2):
                        i = ip + k
                        # emission-order guard: the transpose writing vaug
                        # block i must be EMITTED before this read (the Tile
                        # dep tracker only sees already-emitted writers)
                        while tiles["vt_done"] <= i:
                            yield
                        if zctx["ps"] is None:
                            zctx["ps"] = ps_z.tile(
                                [VW, GW], f32, tag="z", name="z_ps")
                        nc.tensor.matmul(
                            zctx["ps"][:],
                            lhsT=vaug[:, i * VW:(i + 1) * VW],
                            rhs=e_use[:, k * GW:(k + 1) * GW],
                            start=(i == 0),
                            stop=(i == nsk - 1),
                        )
                        yield

                # z runs lag-2 behind scores
                eq = []
                for ip in range(0, nsk, 2):
                    eq.append((ip, emit_scores_pair(ip)))
                    yield
                    # eager flush: the previous group's outproj goes out
                    # right away (keeps the tail short)
                    if pending:
                        yield from flush_outproj()
                    if len(eq) > 2:
                        ip0, e0 = eq.pop(0)
                        yield from emit_z(ip0, e0)
                while eq:
                    ip0, e0 = eq.pop(0)
                    yield from emit_z(ip0, e0)

                z_ps = zctx["ps"]
                zaug = smal.tile([VW, GW], f16, tag="zaug")
                nc.scalar.copy(zaug[:], z_ps[:])
                sums0 = smal.tile([1, GW], f16, tag="sums0")
                nc.vector.tensor_copy(sums0[:], z_ps[DH:VW, :])
                pending.append([zaug, sums0, p, g, wo_sb])

        def interleave(a, b, bpulls=2):
            """Pull a once and b `bpulls` times per cycle until a exhausts;
            b is a shared background stream that survives across calls.
            Front-loading b keeps the PE instruction stream dense (HAM)."""
            a_live = True
            while a_live:
                try:
                    next(a)
                except StopIteration:
                    a_live = False
                for _ in range(bpulls):
                    if b is None:
                        break
                    try:
                        next(b)
                    except StopIteration:
                        b = None
            return b

        def chain(*gens):
            for g in gens:
                yield from g

        tiles = [{} for _ in range(PPC)]
        gens = [gen_proj(p, tiles[p]) for p in range(PPC)]
        # prime pair-0 DMAs, then warm the HAM clock gate with throwaway
        # matmuls on the masks tile (lands ~2us in) so the real projections
        # start at 2.4 GHz instead of ramping from 1.2 until ~36us
        next(gens[0])
        # warmup reads an uninitialized scratch tile: no DMA dependency, so
        # the matmuls issue as soon as the engines come up (~2us) and the HAM
        # clock gate is released before the first real projection
        scratch = consts.tile([P, GW], f16)
        nc.vector.memset(scratch[:], 0)
        warm_ps = psA.tile([DH, GW], f32, tag="u", name="warm_ps")
        for r in range(14):
            nc.tensor.matmul(
                warm_ps[:],
                lhsT=scratch[:, 0:DH],
                rhs=scratch[:],
                start=(r == 0),
                stop=(r == 13),
            )
        # finish the pair-0 head (first K/Q quarter) serially
        for v in gens[0]:
            if v == "head":
                break
        # background: rest of proj(0), then proj(1), proj(2)
        bg = chain(*gens)
        for p in range(PPC):
            # front-load projections during pair 0 (dense PE warms the HAM
            # clock gate); 1:1 after so emitted proj matmuls never get far
            # enough ahead of their input DMAs to head-block the PE queue
            bg = interleave(gen_att(p, tiles[p]), bg, bpulls=1)
        while bg is not None:
            try:
                next(bg)
            except StopIteration:
                bg = None
        while pending:
            for _ in flush_outproj(drain=True):
                pass

    nc.compile()
    return nc


def get_nc(use_bias=False):
    if use_bias not in _NC_CACHE:
        _NC_CACHE[use_bias] = _build_bass(use_bias)
    return _NC_CACHE[use_bias]


def _pairs_for_core(c):
    return [(idx // H, idx % H) for idx in range(c * PPC, (c + 1) * PPC)]


def make_masks():
    # mask[p, (j c)] = 1.0 iff key pos 128*j + p <= query pos c (within block)
    j = np.arange(NG)[None, :, None]
    p = np.arange(P)[:, None, None]
    f = np.arange(GW)[None, None, :]
    return (f >= P * j + p).astype(NP_W).reshape(P, NG * GW)


def _xT_quarters(x, b, h, np_dt):
    # [S, DM] -> [DM, S] -> [P, NQ, NCH, GW] (partition-major quarter blocks)
    xt = x[b, :, h, :].T.astype(np_dt)          # [DM, S]
    xt = xt.reshape(NCH, P, NQ, GW)
    return np.ascontiguousarray(xt.transpose(1, 2, 0, 3))


def make_in_maps(inputs, use_bias):
    xq = np.asarray(inputs["normalized_resid_pre_q"], dtype=np.float32)
    xk = np.asarray(inputs["normalized_resid_pre_k"], dtype=np.float32)
    xv = np.asarray(inputs["normalized_resid_pre_v"], dtype=np.float32)
    W_Q = np.asarray(inputs["W_Q"], dtype=np.float32)
    W_K = np.asarray(inputs["W_K"], dtype=np.float32)
    W_V = np.asarray(inputs["W_V"], dtype=np.float32)
    b_Q = np.asarray(inputs["b_Q"], dtype=np.float32)
    b_K = np.asarray(inputs["b_K"], dtype=np.float32)
    b_V = np.asarray(inputs["b_V"], dtype=np.float32)
    W_O = np.asarray(inputs["W_O"], dtype=np.float32)
    b_O = np.asarray(inputs["b_O"], dtype=np.float32)

    def w_pmajor(W):
        # [DM, DH] -> [NCH, P, DH] -> [P, NCH*DH]
        w = W.astype(NP_W).reshape(NCH, P, DH)
        return np.ascontiguousarray(w.transpose(1, 0, 2)).reshape(P, NCH * DH)

    masks = make_masks()
    onesc = np.ones((P, NSK, 1), NP_W)
    ident64 = np.eye(DH, dtype=NP_W)
    in_maps = []
    for c in range(NCORES):
        pairs = _pairs_for_core(c)
        m = {
            "xqT": np.stack([_xT_quarters(xq, b, h, NP_X8) for b, h in pairs]),
            "xkT": np.stack([_xT_quarters(xk, b, h, NP_X8) for b, h in pairs]),
            "xvT": np.stack([_xT_quarters(xv, b, h, NP_W) for b, h in pairs]),
            "wq": np.stack([w_pmajor(W_Q[h]) for b, h in pairs]),
            "wk": np.stack([w_pmajor(W_K[h]) for b, h in pairs]),
            "wv": np.stack([w_pmajor(W_V[h]) for b, h in pairs]),
            "wo": np.stack(
                [np.concatenate([W_O[h], (b_O / H)[None, :]], axis=0).astype(NP_W)
                 for b, h in pairs]),
            "masks": masks,
            "ones_col": onesc,
            "ident64": ident64,
        }
        if use_bias:
            m["bq"] = np.stack([b_Q[h][None, :].astype(NP_W) for b, h in pairs])
            m["bk"] = np.stack([b_K[h][None, :].astype(NP_W) for b, h in pairs])
            m["bv"] = np.stack([b_V[h][None, :].astype(NP_W) for b, h in pairs])
            m["ones_row"] = np.ones((1, GW), NP_W)
        in_maps.append(m)
    return in_maps


def needs_bias(inputs):
    return any(
        np.any(np.asarray(inputs[k])) for k in ("b_Q", "b_K", "b_V")
    )


def assemble_output(results):
    out = np.empty((B, S, H, DM), np.float32)
    for c in range(NCORES):
        for j, (b, h) in enumerate(_pairs_for_core(c)):
            # outT[j]: [NG, P(m within block), NCH, GW] transposed
            # unnormalized; denoms[j]: [NG, 1, GW]
            o = results[c]["outT"][j].astype(np.float32)
            o = o.reshape(NG, P, NCH, GW).transpose(0, 3, 2, 1).reshape(S, DM)
            d = results[c]["denoms"][j].astype(np.float32).reshape(S, 1)
            out[b, :, h, :] = o / d
    return out


def kernel(**inputs):
    from concourse import bass_utils

    use_bias = needs_bias(inputs)
    nc = get_nc(use_bias)
    in_maps = make_in_maps(inputs, use_bias)
    res = bass_utils.run_bass_kernel_spmd(nc, in_maps, core_ids=list(range(NCORES)))
    return assemble_output(res.results)
